# revision 30
# baseline (speedup 1.0000x reference)
"""Trainium2 Bass kernel for ViT-style LSA attention (sparse_attention).

Reference computation (per batch item):
    qkv = x @ W_qkv.T ; split q,k,v into 12 heads of 64
    dots = (q @ k.T) * scale[h]; diagonal masked to -inf; softmax
    out = (attn @ v) reassembled, then @ W_out.T + b_out

Sharding: data-parallel over batch across 8 NeuronCores (8 items each).

Per-core dataflow (all matmuls bf16 with fp32 PSUM accumulation):
  - x^T and W^T tiles produced on-chip: paired-row DMA loads (fp32) ->
    fp32->bf16 convert (DVE for W_qkv, gpsimd for x/W_out) -> one DMA xbar
    transpose per [128, 768] tile. Staging is INTERLEAVED with compute:
    Tile dependencies follow emission order, so each qkT/V blob is
    emitted right after the staging jobs it needs and waits on nothing
    else (the old stage-everything-first order cost ~50us of PE idle).
  - qk^T = W_qk^T.T @ x^T feature-major, emitted per ITEM (12 feature
    tiles x 197 tokens); the per-head LSA scale is folded into the Q
    tiles during the PSUM->SBUF copy.
  - V natural = x^T.T @ Wv^T token-major, stored per head in 65-wide
    blocks whose 65th column is 1.0 so the PV matmul also produces
    softmax row-sums for free.
  - Attention is SOFTWARE-PIPELINED per item ("weave"): per head pair,
    each j-tile's two S^T matmuls land in a 2-bank PSUM tile, followed
    immediately by one exp (Act) + one mask-multiply covering both
    heads; independent filler blobs (next item's qkT/V projections,
    previous item's output projection, leftover staging jobs) are
    emitted between the S and PV matmuls so the in-order PE queue never
    waits on the Act/DVE softmax chain.
  - out^T[d,i] (+rowsum row 64) per head = V_aug.T @ P^T into a 1-bank
    PSUM tile; normalize with DVE reciprocal + gpsimd partition
    broadcast + DVE multiply into attn^T (feature-major).
  - final = attn^T.T @ W_out^T + b_out (bias via a K=1 ones-row matmul
    in the same PSUM group), PSUM->SBUF copy on DVE, contiguous store.

PSUM budget (8 banks): psS 2x2 + psO 2x1 + psA 2x1.

HW notes (verified the hard way): two matmul accumulation groups may NOT
share a PSUM bank (runtime crash, also with a single start/stop spanning
disjoint ranges); custom-DVE ops (reciprocal_approx_fast) crash at
runtime in this axon environment; DMA cannot read PSUM (bass assert);
issuing a DMA holds the issuing engine's SEQ until the HWDGE queue
grants, so all staging DMAs stay on the sync engine.
"""

from collections import deque
from contextlib import ExitStack

import numpy as np
import ml_dtypes

import concourse.bass as bass
import concourse.bacc as bacc
import concourse.mybir as mybir
import concourse.tile as tile
from concourse import bass_utils, library_config

F32 = mybir.dt.float32
BF16 = mybir.dt.bfloat16
AF = mybir.ActivationFunctionType
ALU = mybir.AluOpType

B, N, D, H, DH = 64, 197, 768, 12, 64
NCORES = 8
BPC = B // NCORES  # batch items per core
KT = D // 128      # 6 contraction tiles of 128
NT_QK = (2 * D) // 128  # 12 feature tiles for q,k

OUT_COPY_DVE = True      # out-projection PSUM->SBUF copy on DVE, not Act
STAGE_CONVERT_POOL = True  # x/W_out staging converts on gpsimd, not DVE
MASK_ON_POOL = False     # diagonal-mask multiply on gpsimd, not DVE


def build_nc(bpc=BPC, repeat=1):
    """Build the kernel. repeat>1 emits the whole body N times back-to-back
    (used only for timing: differencing two repeat counts cancels the fixed
    PJRT dispatch + host<->device transfer overhead)."""
    M = bpc * N  # tokens per core

    nc = bacc.Bacc("TRN2", target_bir_lowering=False, debug=False,
                   num_devices=NCORES)
    x_d = nc.dram_tensor("x", [bpc, N, D], F32, kind="ExternalInput")
    wqkv_d = nc.dram_tensor("w_qkv", [3 * D, D], F32, kind="ExternalInput")
    scale_d = nc.dram_tensor("scale", [H], F32, kind="ExternalInput")
    wout_d = nc.dram_tensor("w_out", [D, D], F32, kind="ExternalInput")
    bout_d = nc.dram_tensor("b_out", [D], F32, kind="ExternalInput")
    out_d = nc.dram_tensor("out", [bpc, N, D], F32, kind="ExternalOutput")

    # Multiplicative diagonal mask for P^T tiles, laid out to match the
    # S tiles: mask4[p, 2*jt+hh, i] = 0 iff i == jt*128 + p.
    mask_np = np.ones((128, 4, N), dtype=ml_dtypes.bfloat16)
    for jt in range(2):
        for p in range(128):
            i = jt * 128 + p
            if i < N:
                mask_np[p, 2 * jt, i] = 0
                mask_np[p, 2 * jt + 1, i] = 0
    mask_d = nc.inline_tensor(mask_np, name="maskc")
    ones_d = nc.inline_tensor(np.ones((1, 128), dtype=ml_dtypes.bfloat16),
                              name="onesc")

    x_flat = x_d[:, :, :].flatten_outer_dims()  # [M, D]
    jtiles = [(0, 128), (1, N - 128)]

    with tile.TileContext(nc) as tc, ExitStack() as es:
        res = es.enter_context(tc.tile_pool(name="res", bufs=1))

        nc.gpsimd.load_library(library_config.attn)

        # ---- resident tiles (allocated once, written by each repeat) ----
        mask_sb = res.tile([128, 4, N], BF16, name="mask_sb")
        ones_bf = res.tile([1, 128], BF16, name="ones_bf")
        scale_row = res.tile([1, H], F32, name="scale_row")
        scale_bc = res.tile([128, H], F32, name="scale_bc")
        scale_bc2 = res.tile([128, KT, 1], F32, name="scale_bc2")
        brow = res.tile([1, D], F32, name="brow")
        bout_bf = res.tile([1, D], BF16, name="bout_bf")
        # token dim padded to 128 so the xbar transpose always moves full
        # [128, 128] tiles (row count must be a multiple of 16); the padding
        # is zero-filled and never read by any matmul.
        M_pad = ((M + 127) // 128) * 128
        v_sb = res.tile([128, bpc, 2, H * 65], BF16, name="v_sb")

        # ---- pools ----
        # The staged-transpose targets are double-buffered (bufs=2) so each
        # repeat's staging DMA overlaps the previous repeat's compute tail;
        # qk^T is pooled PER ITEM (alive ~2 cycles) instead of resident.
        xTp = es.enter_context(tc.tile_pool(name="xTp", bufs=2))
        wqp = es.enter_context(tc.tile_pool(name="wqp", bufs=2))
        wvp = es.enter_context(tc.tile_pool(name="wvp", bufs=2))
        wop = es.enter_context(tc.tile_pool(name="wop", bufs=2))
        qkp = es.enter_context(tc.tile_pool(name="qkp", bufs=3))
        stg = es.enter_context(tc.tile_pool(name="stg", bufs=4))
        stgb = es.enter_context(tc.tile_pool(name="stgb", bufs=2))
        # PSUM pools: 8 banks total = psS 1x2 + psO 4x1 + psA 2x1.
        psA = es.enter_context(tc.tile_pool(name="psA", bufs=2, space="PSUM"))
        psS = es.enter_context(tc.tile_pool(name="psS", bufs=1, space="PSUM"))
        psO = es.enter_context(tc.tile_pool(name="psO", bufs=4, space="PSUM"))
        ptp = es.enter_context(tc.tile_pool(name="ptp", bufs=4))
        rcp = es.enter_context(tc.tile_pool(name="rcp", bufs=4))
        bcp = es.enter_context(tc.tile_pool(name="bcp", bufs=4))
        atp = es.enter_context(tc.tile_pool(name="atp", bufs=2))
        osp = es.enter_context(tc.tile_pool(name="osp", bufs=2))

        def stage_load(src_ap, nrows, n2):
            """Phase 1 of a staging job: paired-row DMA load (fp32)."""
            t_f = stg.tile([128, 2, D], F32, tag="stg", name="t_f")
            if nrows < n2 * 128:
                nc.vector.memset(t_f, 0.0)
            if nrows > 128:
                nc.sync.dma_start(
                    t_f[:, :2], src_ap.rearrange("(t p) f -> p t f", p=128))
            else:
                nc.sync.dma_start(t_f[:nrows, 0], src_ap)
            return t_f

        def stage_rest(t_f, dsts, n2):
            """Phase 2: fp32->bf16 convert (gpsimd) + one xbar transpose
            per [128, 768] row-tile. Emitted a couple of pipeline slots
            after phase 1 so the convert never reaches the Pool queue head
            before its DMA load has finished (every engine stream is
            in-order: a waiting op stalls everything behind it)."""
            t_b = stgb.tile([128, 2, D], BF16, tag="stgb", name="t_b")
            nc.gpsimd.tensor_copy(t_b[:, :n2], t_f[:, :n2])
            for t, dst in enumerate(dsts):
                if dst is None:
                    continue
                # one xbar transpose per [128, 768] tile: 3D dst gets row
                # kt*128+p at [p, kt, m] (sim-verified)
                nc.sync.dma_start_transpose(dst, t_b[:, t])

        def pair_jobs(src_rows, total_rows, dst_fn):
            """Two-phase (load, rest) emitter pairs, 256 rows apiece."""
            jobs = []
            r0 = 0
            while r0 < total_rows:
                rows = min(256, total_rows - r0)
                if rows < 256:
                    rows = min(128, rows)  # singles for the tail
                d0 = dst_fn(r0)
                d1 = dst_fn(r0 + 128) if rows > 128 else None
                box = {}

                def load(box=box, s=src_rows(r0, rows), n=rows,
                         n2=(2 if rows > 128 else 1)):
                    box["t"] = stage_load(s, n, n2)

                def rest(box=box, d=(d0, d1), n2=(2 if rows > 128 else 1)):
                    stage_rest(box.pop("t"), d, n2)

                jobs.append((load, rest))
                r0 += rows
            return jobs

        def emit_preamble():
            """Constant setup, emitted ONCE (not per repeat): these tiles
            hold input-derived constants that no repeat overwrites. A
            per-repeat re-DMA would sem-wait on the previous repeat's
            readers while holding the sync SEQ, stalling the whole DMA
            queue at each repeat seam."""
            nc.sync.dma_start(mask_sb, mask_d[:, :, :])
            nc.sync.dma_start(ones_bf, ones_d[:, :])
            nc.sync.dma_start(scale_row, scale_d[None, :])
            nc.gpsimd.partition_broadcast(scale_bc, scale_row)
            for nt in range(KT):
                for hh in range(2):
                    nc.vector.tensor_copy(
                        scale_bc2[hh * 64:(hh + 1) * 64, nt],
                        scale_bc[hh * 64:(hh + 1) * 64,
                                 2 * nt + hh:2 * nt + hh + 1])
            nc.sync.dma_start(brow, bout_d[None, :])
            nc.vector.tensor_copy(bout_bf, brow)
            nc.vector.memset(
                v_sb.rearrange("p b j (h e) -> p b j h e",
                               e=65)[:, :, :, :, 64:65], 1.0)

        cur = {}  # per-repeat staged tiles (rotated for cross-rep overlap)

        def qkT_item_blobs(b, qk_t):
            """12 filler blobs: qk^T feature tiles for item b's tokens."""
            t0 = b * N
            blobs = []
            for nt in range(NT_QK):
                def blob(nt=nt):
                    ps = psA.tile([128, 512], F32, tag="psA", name="ps_qk")
                    for kt in range(KT):
                        nc.tensor.matmul(
                            ps[:, :N],
                            lhsT=cur["wqkT"][:, kt, nt * 128:(nt + 1) * 128],
                            rhs=cur["xT"][:, kt, t0:t0 + N],
                            start=(kt == 0), stop=(kt == KT - 1))
                    if nt < KT:  # Q tiles: fold in the per-head LSA scale
                        nc.scalar.mul(qk_t[:, nt, :], ps[:, :N],
                                      scale_bc2[:, nt])
                    else:
                        nc.scalar.copy(qk_t[:, nt, :], ps[:, :N])
                blobs.append(blob)
            return blobs

        def v_item_blobs(b):
            """4 filler blobs: token-major V (+ones col) for item b,
            ordered (jt0,nch0), (jt1,nch0), (jt0,nch1), (jt1,nch1)."""
            blobs = []
            for nch in range(2):
                for jt, jsz in jtiles:
                    def blob(jt=jt, jsz=jsz, nch=nch):
                        ps = psA.tile([128, 512], F32, tag="psA", name="ps_v")
                        for kt in range(KT):
                            nc.tensor.matmul(
                                ps[:jsz, :384],
                                lhsT=cur["xT"][:, kt,
                                               b * N + jt * 128:
                                               b * N + jt * 128 + jsz],
                                rhs=cur["wvT"][:, kt,
                                               nch * 384:(nch + 1) * 384],
                                start=(kt == 0), stop=(kt == KT - 1))
                        dst = v_sb[:jsz, b, jt].rearrange(
                            "p (h e) -> p h e",
                            e=65)[:, nch * 6:(nch + 1) * 6, 0:64]
                        nc.scalar.copy(
                            dst,
                            ps[:jsz, :384].rearrange("p (h e) -> p h e", e=64))
                    blobs.append(blob)
            return blobs

        def outproj_blobs(b, attnT):
            """4 filler blobs: output projection halves for item b."""
            blobs = []
            for jt, jsz in jtiles:
                cell = []

                def half(jt, jsz, nch, cell):
                    ps = psA.tile([128, 512], F32, tag="psA", name="ps_o")
                    for ft in range(KT):
                        nc.tensor.matmul(
                            ps[:jsz, :384],
                            lhsT=attnT[:, ft, jt * 128:jt * 128 + jsz],
                            rhs=cur["woT"][:, ft, nch * 384:(nch + 1) * 384],
                            start=(ft == 0), stop=False)
                    nc.tensor.matmul(
                        ps[:jsz, :384], lhsT=ones_bf[0:1, :jsz],
                        rhs=bout_bf[0:1, nch * 384:(nch + 1) * 384],
                        start=False, stop=True)
                    if OUT_COPY_DVE:
                        nc.vector.tensor_copy(
                            cell[0][:jsz, nch * 384:(nch + 1) * 384],
                            ps[:jsz, :384])
                    else:
                        nc.scalar.copy(
                            cell[0][:jsz, nch * 384:(nch + 1) * 384],
                            ps[:jsz, :384])

                def blob0(jt=jt, jsz=jsz, cell=cell):
                    cell.append(osp.tile([128, D], F32, tag="osb", name="osb"))
                    half(jt, jsz, 0, cell)

                def blob1(jt=jt, jsz=jsz, cell=cell):
                    half(jt, jsz, 1, cell)
                    # one fully-contiguous [jsz, 768] store per (item, j-tile)
                    nc.sync.dma_start(out_d[b, jt * 128:jt * 128 + jsz, :],
                                      cell[0][:jsz])
                blobs += [blob0, blob1]
            return blobs

        def emit_attn_weave(b, qk_t, fill_proj, fill_out, sprinkle):
            """Attention for item b, software-pipelined: filler blobs are
            popped between the S and PV matmuls of each head pair so the
            in-order PE stream never waits on the Act/DVE softmax chain.
            `sprinkle` holds staging jobs (DMA-side) to emit along the way."""
            attnT = atp.tile([128, KT, N], BF16, tag="attnT", name="attnT")

            def pop(dq1, dq2):
                if dq1:
                    dq1.popleft()()
                elif dq2:
                    dq2.popleft()()

            meng = nc.gpsimd if MASK_ON_POOL else nc.vector
            HP = H // 2
            pts, ots, rcs, bcs = {}, {}, {}, {}
            # THE SOFTMAX CHAIN IS STAGE-LAGGED: every engine stream is
            # strictly in-order (a sem-wait at the queue head stalls all
            # later ops on that engine), so each link of the chain runs a
            # full stage after its producer: S/exp @ s, PV+recip @ s+1,
            # bcast @ s+1 (end), normalize-mult @ s+2. By the time an op
            # reaches its queue head, its inputs are already complete and
            # no engine ever blocks.
            for s in range(HP + 2):
                if s < HP:
                    # Per j-tile: both heads' S^T matmuls into the 2-bank
                    # tile, then ONE exp + ONE mask multiply at 2N width.
                    for jt, jsz in jtiles:
                        st2 = psS.tile([128, 2, N], F32, tag="psS",
                                       name="st2",
                                       padded_shape=[128, 2, 512])
                        for hh in range(2):
                            pb = hh * 64
                            qa = qk_t[pb:pb + 64, s, :]
                            ka = qk_t[pb:pb + 64, 6 + s, :]
                            nc.tensor.matmul(
                                st2[:jsz, hh],
                                lhsT=ka[:, jt * 128:jt * 128 + jsz],
                                rhs=qa, start=True, stop=True)
                        pt = ptp.tile([128, 2, N], BF16, tag="pt", name="pt2")
                        nc.scalar.activation(pt[:jsz], st2[:jsz], AF.Exp)
                        pts.setdefault(s, []).append(pt)
                if 1 <= s <= HP:
                    hp = s - 1
                    for hh in range(2):
                        h = 2 * hp + hh
                        ot = psO.tile([65, 512], F32, tag="psO", name="ot")
                        for jt, jsz in jtiles:
                            nc.tensor.matmul(
                                ot[:, :N],
                                lhsT=v_sb[:jsz, b, jt, h * 65:h * 65 + 65],
                                rhs=pts[hp][jt][:jsz, hh],
                                start=(jt == 0), stop=(jt == 1))
                        ots[(hp, hh)] = ot
                pop(fill_proj, fill_out)
                if s < HP:
                    for jt, jsz in jtiles:
                        meng.tensor_tensor(pts[s][jt][:jsz], pts[s][jt][:jsz],
                                           mask_sb[:jsz, 2 * jt:2 * jt + 2],
                                           op=ALU.mult)
                if 1 <= s <= HP:
                    hp = s - 1
                    for hh in range(2):
                        rc = rcp.tile([1, N], F32, tag="rc", name="rc")
                        nc.vector.reciprocal(rc, ots[(hp, hh)][64:65, :N])
                        rcs[(hp, hh)] = rc
                if 2 <= s:
                    hp = s - 2
                    for hh in range(2):
                        nc.vector.tensor_tensor(
                            attnT[hh * 64:hh * 64 + 64, hp, :],
                            ots.pop((hp, hh))[0:64, :N],
                            bcs.pop((hp, hh)), op=ALU.mult)
                    pts.pop(hp, None)
                if s >= 2:
                    pop(fill_out, fill_proj)
                else:
                    pop(fill_proj, fill_out)
                pop(fill_proj, fill_out)
                if 1 <= s <= HP:
                    hp = s - 1
                    for hh in range(2):
                        bc = bcp.tile([64, N], F32, tag="bc", name="bc")
                        nc.gpsimd.partition_broadcast(bc, rcs.pop((hp, hh)))
                        bcs[(hp, hh)] = bc
                if sprinkle:
                    sprinkle.popleft()()
            return attnT

        def make_stage():
            """Allocate one repeat's staged tiles and build its 19 staging
            job emitters (not yet emitted)."""
            tiles = {
                "xT": xTp.tile([128, KT, M_pad], BF16, tag="xT", name="xT"),
                "wqkT": wqp.tile([128, KT, 2 * D], BF16, tag="wqkT",
                                 name="wqkT"),
                "wvT": wvp.tile([128, KT, D], BF16, tag="wvT", name="wvT"),
                "woT": wop.tile([128, KT, D], BF16, tag="woT", name="woT"),
            }
            jobs = {
                "x": pair_jobs(lambda r0, rows: x_flat[r0:r0 + rows, :],
                               M, lambda r0: tiles["xT"][:, :, r0:r0 + 128]),
                "wqk": pair_jobs(lambda r0, rows: wqkv_d[r0:r0 + rows, :],
                                 2 * D,
                                 lambda r0: tiles["wqkT"][:, :, r0:r0 + 128]),
                "wv": pair_jobs(
                    lambda r0, rows: wqkv_d[2 * D + r0:2 * D + r0 + rows, :],
                    D, lambda r0: tiles["wvT"][:, :, r0:r0 + 128]),
                "wout": pair_jobs(lambda r0, rows: wout_d[r0:r0 + rows, :],
                                  D, lambda r0: tiles["woT"][:, :, r0:r0 + 128]),
            }
            return tiles, jobs

        def emit_rep_body(tiles, own_jobs, next_jobs):
            """One repeat's compute. own_jobs is set only for the FIRST
            repeat (cold staging interleaved with the prologue); later
            repeats find their tiles already staged, because each repeat
            sprinkles the NEXT repeat's staging jobs through its cycles
            (the sync SEQ is in-order, so only emission-time interleaving
            can overlap staging DMA with the previous repeat's compute)."""
            cur.clear()
            cur.update(tiles)
            qk_tiles = {}

            def qk_tile(b):
                if b not in qk_tiles:
                    qk_tiles[b] = qkp.tile([128, NT_QK, N], BF16, tag="qk",
                                           name="qk_t")
                return qk_tiles[b]

            qk0 = qkT_item_blobs(0, qk_tile(0))
            v0 = v_item_blobs(0)

            def run_job(j):
                j[0]()
                j[1]()

            if own_jobs is not None:
                # ---- cold prologue: stage and compute hand in hand ----
                xjobs, wqkjobs = own_jobs["x"], own_jobs["wqk"]
                wvjobs, woutjobs = own_jobs["wv"], own_jobs["wout"]
                run_job(wqkjobs[0])
                run_job(xjobs[0])
                run_job(xjobs[1])
                qk0[0]()
                qk0[1]()
                for j in range(1, 6):
                    run_job(wqkjobs[j])  # cols 256j..-: feature tiles 2j,2j+1
                    qk0[2 * j]()
                    qk0[2 * j + 1]()
                run_job(wvjobs[0])
                run_job(wvjobs[1])  # wv rows 0..511: nch0 ready
                v0[0]()
                v0[1]()
                run_job(wvjobs[2])  # nch1 ready
                run_job(xjobs[2])
                v0[2]()
                v0[3]()
                run_job(xjobs[3])
                own_left = [xjobs[4], *woutjobs, *xjobs[5:]]
            else:
                for f in qk0:
                    f()
                for f in v0:
                    f()
                own_left = []
            # sprinkle: leftover cold staging first, then the next repeat's
            # staging jobs. Loads run LAG slots ahead of their convert +
            # transpose phase so no engine queue ever parks on a DMA.
            nxt = []
            if next_jobs is not None:
                nxt = [*next_jobs["wqk"], *next_jobs["x"][:2],
                       *next_jobs["wv"], *next_jobs["x"][2:],
                       *next_jobs["wout"]]
            todo = own_left + nxt
            LAG = 2
            flat = []
            pending = deque()
            for ld, rs in todo:
                flat.append(ld)
                pending.append(rs)
                if len(pending) > LAG:
                    flat.append(pending.popleft())
            flat.extend(pending)
            sprinkle = deque(flat)
            prev_attnT = None
            for b in range(bpc):
                fp = deque()
                if b + 1 < bpc:
                    fp.extend(qkT_item_blobs(b + 1, qk_tile(b + 1)))
                    fp.extend(v_item_blobs(b + 1))
                fo = deque()
                if b >= 1:
                    fo.extend(outproj_blobs(b - 1, prev_attnT))
                new_attnT = emit_attn_weave(b, qk_tile(b), fp, fo, sprinkle)
                for f in fp:
                    f()
                for f in fo:
                    f()
                prev_attnT = new_attnT
            for f in outproj_blobs(bpc - 1, prev_attnT):
                f()

        emit_preamble()
        tiles, jobs = make_stage()
        own = jobs
        for _rep in range(repeat):
            if _rep + 1 < repeat:
                ntiles, njobs = make_stage()
            else:
                ntiles, njobs = None, None
            emit_rep_body(tiles, own, njobs)
            tiles, own = ntiles, None

    nc.compile()
    return nc


_NC_CACHE = {}


def _get_nc(bpc=BPC, repeat=1):
    key = (bpc, repeat)
    if key not in _NC_CACHE:
        _NC_CACHE[key] = build_nc(bpc, repeat)
    return _NC_CACHE[key]


def kernel(x, W_qkv, scale, W_out, b_out, _trace=False):
    x = np.ascontiguousarray(np.asarray(x, dtype=np.float32))
    W_qkv = np.ascontiguousarray(np.asarray(W_qkv, dtype=np.float32))
    scale = np.ascontiguousarray(np.asarray(scale, dtype=np.float32))
    W_out = np.ascontiguousarray(np.asarray(W_out, dtype=np.float32))
    b_out = np.ascontiguousarray(np.asarray(b_out, dtype=np.float32))

    nc = _get_nc()
    in_maps = [{
        "x": x[c * BPC:(c + 1) * BPC],
        "w_qkv": W_qkv,
        "scale": scale,
        "w_out": W_out,
        "b_out": b_out,
    } for c in range(NCORES)]
    try:
        res = bass_utils.run_bass_kernel_spmd(
            nc, in_maps, core_ids=list(range(NCORES)), trace=_trace)
    except ModuleNotFoundError:
        # axon NTFF profiling hook unavailable in this container
        res = bass_utils.run_bass_kernel_spmd(
            nc, in_maps, core_ids=list(range(NCORES)), trace=False)
    out = np.concatenate([res.results[c]["out"] for c in range(NCORES)], axis=0)
    if _trace:
        return out, res
    return out


# revision 39
# speedup vs baseline: 1.1832x; 1.1832x over previous
"""Trainium2 Bass kernel for ViT-style LSA attention (sparse_attention).

Reference computation (per batch item):
    qkv = x @ W_qkv.T ; split q,k,v into 12 heads of 64
    dots = (q @ k.T) * scale[h]; diagonal masked to -inf; softmax
    out = (attn @ v) reassembled, then @ W_out.T + b_out

Sharding: data-parallel over batch across 8 NeuronCores (8 items each).

Per-core dataflow (all matmuls bf16 with fp32 PSUM accumulation):
  - x^T and W^T tiles produced on-chip: paired-row DMA loads (fp32) ->
    fp32->bf16 convert (DVE for W_qkv, gpsimd for x/W_out) -> one DMA xbar
    transpose per [128, 768] tile. Staging is INTERLEAVED with compute:
    Tile dependencies follow emission order, so each qkT/V blob is
    emitted right after the staging jobs it needs and waits on nothing
    else (the old stage-everything-first order cost ~50us of PE idle).
  - qk^T = W_qk^T.T @ x^T feature-major, emitted per ITEM (12 feature
    tiles x 197 tokens); the per-head LSA scale is folded into the Q
    tiles during the PSUM->SBUF copy.
  - V natural = x^T.T @ Wv^T token-major, stored per head in 65-wide
    blocks whose 65th column is 1.0 so the PV matmul also produces
    softmax row-sums for free.
  - Attention is SOFTWARE-PIPELINED per item ("weave"): per head pair,
    each j-tile's two S^T matmuls land in a 2-bank PSUM tile, followed
    immediately by one exp (Act) + one mask-multiply covering both
    heads; independent filler blobs (next item's qkT/V projections,
    previous item's output projection, leftover staging jobs) are
    emitted between the S and PV matmuls so the in-order PE queue never
    waits on the Act/DVE softmax chain.
  - out^T[d,i] (+rowsum row 64) per head = V_aug.T @ P^T into a 1-bank
    PSUM tile; normalize with DVE reciprocal + gpsimd partition
    broadcast + DVE multiply into attn^T (feature-major).
  - final = attn^T.T @ W_out^T + b_out (bias via a K=1 ones-row matmul
    in the same PSUM group), PSUM->SBUF copy on DVE, contiguous store.

PSUM budget (8 banks): psS 2x2 + psO 2x1 + psA 2x1.

HW notes (verified the hard way): two matmul accumulation groups may NOT
share a PSUM bank (runtime crash, also with a single start/stop spanning
disjoint ranges); custom-DVE ops (reciprocal_approx_fast) crash at
runtime in this axon environment; DMA cannot read PSUM (bass assert);
issuing a DMA holds the issuing engine's SEQ until the HWDGE queue
grants, so all staging DMAs stay on the sync engine.
"""

from collections import deque
from contextlib import ExitStack

import numpy as np
import ml_dtypes

import concourse.bass as bass
import concourse.bacc as bacc
import concourse.mybir as mybir
import concourse.tile as tile
from concourse import bass_utils, library_config

F32 = mybir.dt.float32
BF16 = mybir.dt.bfloat16
AF = mybir.ActivationFunctionType
ALU = mybir.AluOpType

B, N, D, H, DH = 64, 197, 768, 12, 64
NCORES = 8
BPC = B // NCORES  # batch items per core
KT = D // 128      # 6 contraction tiles of 128
NT_QK = (2 * D) // 128  # 12 feature tiles for q,k

OUT_COPY_DVE = True      # out-projection PSUM->SBUF copy on DVE, not Act
STAGE_CONVERT_POOL = True  # x/W_out staging converts on gpsimd, not DVE
MASK_ON_POOL = False     # diagonal-mask multiply on DVE (Pool is ~3x slower)
N_PAD = 208              # 197 tokens padded to a multiple of 16 for the xbar


def build_nc(bpc=BPC, repeat=1):
    """Build the kernel. repeat>1 emits the whole body N times back-to-back
    (used only for timing: differencing two repeat counts cancels the fixed
    PJRT dispatch + host<->device transfer overhead)."""
    M = bpc * N  # tokens per core

    nc = bacc.Bacc("TRN2", target_bir_lowering=False, debug=False,
                   num_devices=NCORES)
    x_d = nc.dram_tensor("x", [bpc, N, D], F32, kind="ExternalInput")
    wqkv_d = nc.dram_tensor("w_qkv", [3 * D, D], F32, kind="ExternalInput")
    scale_d = nc.dram_tensor("scale", [H], F32, kind="ExternalInput")
    wout_d = nc.dram_tensor("w_out", [D, D], F32, kind="ExternalInput")
    bout_d = nc.dram_tensor("b_out", [D], F32, kind="ExternalInput")
    out_d = nc.dram_tensor("out", [bpc, N, D], F32, kind="ExternalOutput")

    # Multiplicative diagonal mask for P^T tiles, laid out to match the
    # S tiles: mask4[p, 2*jt+hh, i] = 0 iff i == jt*128 + p.
    mask_np = np.ones((128, 4, N), dtype=ml_dtypes.bfloat16)
    for jt in range(2):
        for p in range(128):
            i = jt * 128 + p
            if i < N:
                mask_np[p, 2 * jt, i] = 0
                mask_np[p, 2 * jt + 1, i] = 0
    mask_d = nc.inline_tensor(mask_np, name="maskc")
    ones_d = nc.inline_tensor(np.ones((1, 128), dtype=ml_dtypes.bfloat16),
                              name="onesc")

    x_flat = x_d[:, :, :].flatten_outer_dims()  # [M, D]
    jtiles = [(0, 128), (1, N - 128)]

    with tile.TileContext(nc) as tc, ExitStack() as es:
        res = es.enter_context(tc.tile_pool(name="res", bufs=1))

        nc.gpsimd.load_library(library_config.attn)

        # ---- resident tiles (allocated once, written by each repeat) ----
        mask_sb = res.tile([128, 4, N], BF16, name="mask_sb")
        ones_bf = res.tile([1, 128], BF16, name="ones_bf")
        scale_row = res.tile([1, H], F32, name="scale_row")
        scale_bc = res.tile([128, H], F32, name="scale_bc")
        scale_bc2 = res.tile([128, KT, 1], F32, name="scale_bc2")
        brow = res.tile([1, D], F32, name="brow")
        bout_bf = res.tile([1, D], BF16, name="bout_bf")
        # token dim padded to 128 so the xbar transpose always moves full
        # [128, 128] tiles (row count must be a multiple of 16); the padding
        # is zero-filled and never read by any matmul.
        M_pad = ((M + 127) // 128) * 128
        v_sb = res.tile([128, bpc, 2, H * 65], BF16, name="v_sb")

        # ---- pools ----
        # The staged-transpose targets are double-buffered (bufs=2) so each
        # repeat's staging DMA overlaps the previous repeat's compute tail;
        # qk^T is pooled PER ITEM (alive ~2 cycles) instead of resident.
        xTp = es.enter_context(tc.tile_pool(name="xTp", bufs=2))
        wqp = es.enter_context(tc.tile_pool(name="wqp", bufs=2))
        wvp = es.enter_context(tc.tile_pool(name="wvp", bufs=2))
        wop = es.enter_context(tc.tile_pool(name="wop", bufs=2))
        qkp = es.enter_context(tc.tile_pool(name="qkp", bufs=3))
        stg = es.enter_context(tc.tile_pool(name="stg", bufs=3))
        stgb = es.enter_context(tc.tile_pool(name="stgb", bufs=2))
        # PSUM pools: 8 banks total = psS 1x2 + psO 4x1 + psA 2x1.
        psA = es.enter_context(tc.tile_pool(name="psA", bufs=2, space="PSUM"))
        psS = es.enter_context(tc.tile_pool(name="psS", bufs=1, space="PSUM"))
        psO = es.enter_context(tc.tile_pool(name="psO", bufs=4, space="PSUM"))
        ptp = es.enter_context(tc.tile_pool(name="ptp", bufs=4))
        rcp = es.enter_context(tc.tile_pool(name="rcp", bufs=4))
        akp = es.enter_context(tc.tile_pool(name="akp", bufs=4))
        atp = es.enter_context(tc.tile_pool(name="atp", bufs=2))
        osp = es.enter_context(tc.tile_pool(name="osp", bufs=2))

        def stage_load(src_ap, nrows, n2):
            """Phase 1 of a staging job: paired-row DMA load (fp32)."""
            t_f = stg.tile([128, 2, D], F32, tag="stg", name="t_f")
            if nrows < n2 * 128:
                nc.vector.memset(t_f, 0.0)
            if nrows > 128:
                nc.sync.dma_start(
                    t_f[:, :2], src_ap.rearrange("(t p) f -> p t f", p=128))
            else:
                nc.sync.dma_start(t_f[:nrows, 0], src_ap)
            return t_f

        def stage_rest(t_f, dsts, n2):
            """Phase 2: fp32->bf16 convert (gpsimd) + one xbar transpose
            per [128, 768] row-tile. Emitted a couple of pipeline slots
            after phase 1 so the convert never reaches the Pool queue head
            before its DMA load has finished (every engine stream is
            in-order: a waiting op stalls everything behind it)."""
            t_b = stgb.tile([128, 2, D], BF16, tag="stgb", name="t_b")
            nc.gpsimd.tensor_copy(t_b[:, :n2], t_f[:, :n2])
            for t, dst in enumerate(dsts):
                if dst is None:
                    continue
                # one xbar transpose per [128, 768] tile: 3D dst gets row
                # kt*128+p at [p, kt, m] (sim-verified)
                nc.sync.dma_start_transpose(dst, t_b[:, t])

        def pair_jobs(src_rows, total_rows, dst_fn):
            """Two-phase (load, rest) emitter pairs, 256 rows apiece."""
            jobs = []
            r0 = 0
            while r0 < total_rows:
                rows = min(256, total_rows - r0)
                if rows < 256:
                    rows = min(128, rows)  # singles for the tail
                d0 = dst_fn(r0)
                d1 = dst_fn(r0 + 128) if rows > 128 else None
                box = {}

                def load(box=box, s=src_rows(r0, rows), n=rows,
                         n2=(2 if rows > 128 else 1)):
                    box["t"] = stage_load(s, n, n2)

                def rest(box=box, d=(d0, d1), n2=(2 if rows > 128 else 1)):
                    stage_rest(box.pop("t"), d, n2)

                jobs.append((load, rest))
                r0 += rows
            return jobs

        def emit_preamble():
            """Constant setup, emitted ONCE (not per repeat): these tiles
            hold input-derived constants that no repeat overwrites. A
            per-repeat re-DMA would sem-wait on the previous repeat's
            readers while holding the sync SEQ, stalling the whole DMA
            queue at each repeat seam."""
            nc.sync.dma_start(mask_sb, mask_d[:, :, :])
            nc.sync.dma_start(ones_bf, ones_d[:, :])
            nc.sync.dma_start(scale_row, scale_d[None, :])
            nc.gpsimd.partition_broadcast(scale_bc, scale_row)
            for nt in range(KT):
                for hh in range(2):
                    nc.vector.tensor_copy(
                        scale_bc2[hh * 64:(hh + 1) * 64, nt],
                        scale_bc[hh * 64:(hh + 1) * 64,
                                 2 * nt + hh:2 * nt + hh + 1])
            nc.sync.dma_start(brow, bout_d[None, :])
            nc.vector.tensor_copy(bout_bf, brow)
            nc.vector.memset(
                v_sb.rearrange("p b j (h e) -> p b j h e",
                               e=65)[:, :, :, :, 64:65], 1.0)

        cur = {}  # per-repeat staged tiles (rotated for cross-rep overlap)

        def qkT_item_blobs(b, qk_t):
            """12 filler blobs: qk^T feature tiles for item b's tokens."""
            t0 = b * N
            blobs = []
            for nt in range(NT_QK):
                def blob(nt=nt):
                    ps = psA.tile([128, 512], F32, tag="psA", name="ps_qk")
                    for kt in range(KT):
                        nc.tensor.matmul(
                            ps[:, :N],
                            lhsT=cur["wqkT"][:, kt, nt * 128:(nt + 1) * 128],
                            rhs=cur["xT"][:, kt, t0:t0 + N],
                            start=(kt == 0), stop=(kt == KT - 1))
                    if nt < KT:  # Q tiles: fold in the per-head LSA scale
                        nc.scalar.mul(qk_t[:, nt, :], ps[:, :N],
                                      scale_bc2[:, nt])
                    else:
                        nc.scalar.copy(qk_t[:, nt, :], ps[:, :N])
                blobs.append(blob)
            return blobs

        def v_item_blobs(b):
            """4 filler blobs: token-major V (+ones col) for item b,
            ordered (jt0,nch0), (jt1,nch0), (jt0,nch1), (jt1,nch1)."""
            blobs = []
            for nch in range(2):
                for jt, jsz in jtiles:
                    def blob(jt=jt, jsz=jsz, nch=nch):
                        ps = psA.tile([128, 512], F32, tag="psA", name="ps_v")
                        for kt in range(KT):
                            nc.tensor.matmul(
                                ps[:jsz, :384],
                                lhsT=cur["xT"][:, kt,
                                               b * N + jt * 128:
                                               b * N + jt * 128 + jsz],
                                rhs=cur["wvT"][:, kt,
                                               nch * 384:(nch + 1) * 384],
                                start=(kt == 0), stop=(kt == KT - 1))
                        dst = v_sb[:jsz, b, jt].rearrange(
                            "p (h e) -> p h e",
                            e=65)[:, nch * 6:(nch + 1) * 6, 0:64]
                        nc.scalar.copy(
                            dst,
                            ps[:jsz, :384].rearrange("p (h e) -> p h e", e=64))
                    blobs.append(blob)
            return blobs

        def outproj_blobs(b, attnT):
            """4 filler blobs: output projection halves for item b."""
            blobs = []
            for jt, jsz in jtiles:
                cell = []

                def half(jt, jsz, nch, cell):
                    ps = psA.tile([128, 512], F32, tag="psA", name="ps_o")
                    for ft in range(KT):
                        nc.tensor.matmul(
                            ps[:jsz, :384],
                            lhsT=attnT[:, ft, jt * 128:jt * 128 + jsz],
                            rhs=cur["woT"][:, ft, nch * 384:(nch + 1) * 384],
                            start=(ft == 0), stop=False)
                    nc.tensor.matmul(
                        ps[:jsz, :384], lhsT=ones_bf[0:1, :jsz],
                        rhs=bout_bf[0:1, nch * 384:(nch + 1) * 384],
                        start=False, stop=True)
                    if OUT_COPY_DVE:
                        nc.vector.tensor_copy(
                            cell[0][:jsz, nch * 384:(nch + 1) * 384],
                            ps[:jsz, :384])
                    else:
                        nc.scalar.copy(
                            cell[0][:jsz, nch * 384:(nch + 1) * 384],
                            ps[:jsz, :384])

                def blob0(jt=jt, jsz=jsz, cell=cell):
                    cell.append(osp.tile([128, D], F32, tag="osb", name="osb"))
                    half(jt, jsz, 0, cell)

                def blob1(jt=jt, jsz=jsz, cell=cell):
                    half(jt, jsz, 1, cell)
                    # one fully-contiguous [jsz, 768] store per (item, j-tile)
                    nc.sync.dma_start(out_d[b, jt * 128:jt * 128 + jsz, :],
                                      cell[0][:jsz])
                blobs += [blob0, blob1]
            return blobs

        def transpose_attn(toks):
            """One xbar transpose pass: token-major attn [i, f] (two row
            tiles, the second padded to 80 rows) -> feature-major attn^T
            [f-part, kt, i] for the output projection. Columns 197..207
            receive garbage from the pad rows and are never read."""
            attnT = atp.tile([128, KT, N_PAD], BF16, tag="attnT",
                             name="attnT")
            # issued from the Act DGE queue (only SP/Act can drive HWDGE):
            # skips the load/store-laden sync queue so the out-projection
            # isn't stuck behind staging transfers.
            nc.scalar.dma_start_transpose(attnT[:, :, 0:128], toks[0])
            nc.scalar.dma_start_transpose(attnT[:, :, 128:N_PAD],
                                          toks[1][0:80])
            return attnT

        def emit_attn_weave(b, qk_t, fill_proj, fill_out, sprinkle):
            """Attention for item b, software-pipelined: filler blobs are
            popped between the S and PV matmuls of each head pair so the
            in-order PE stream never waits on the Act/DVE softmax chain.
            `sprinkle` holds staging jobs (DMA-side) to emit along the way."""

            def pop(dq1, dq2):
                if dq1:
                    dq1.popleft()()
                elif dq2:
                    dq2.popleft()()

            meng = nc.gpsimd if MASK_ON_POOL else nc.vector
            HP = H // 2
            itiles = [(0, 128), (1, N - 128)]
            pts, ots = {}, {}
            # TOKEN-MAJOR PV: out[i, d] = P^T.T @ V_aug puts query tokens on
            # the PSUM partitions, so the softmax row-sum (65th column) is a
            # PER-PARTITION scalar: normalize is a [jsz,1] reciprocal + a
            # per-partition tensor_scalar multiply — no partition broadcast,
            # no wide multiplies. The chain is stage-lagged (every engine
            # stream is in-order, so a sem-wait at the queue head stalls all
            # later ops): S/exp @ s, mask @ s (after a filler), PV @ s+1,
            # recip+mul @ s+1 (after a filler). attn lands token-major in
            # SBUF; one xbar transpose per item (next cycle) rebuilds the
            # feature-major attn^T that the output projection consumes.
            toks = [akp.tile([128, D], BF16, tag="tok", name="tok")
                    for _ in range(2)]
            for s in range(HP + 1):
                if s < HP:
                    # Per j-tile: both heads' S^T matmuls into the 2-bank
                    # tile, then ONE exp at 2N width.
                    for jt, jsz in jtiles:
                        st2 = psS.tile([128, 2, N], F32, tag="psS",
                                       name="st2",
                                       padded_shape=[128, 2, 512])
                        for hh in range(2):
                            pb = hh * 64
                            qa = qk_t[pb:pb + 64, s, :]
                            ka = qk_t[pb:pb + 64, 6 + s, :]
                            nc.tensor.matmul(
                                st2[:jsz, hh],
                                lhsT=ka[:, jt * 128:jt * 128 + jsz],
                                rhs=qa, start=True, stop=True)
                        pt = ptp.tile([128, 2, N], BF16, tag="pt", name="pt2")
                        nc.scalar.activation(pt[:jsz], st2[:jsz], AF.Exp)
                        pts.setdefault(s, []).append(pt)
                if 1 <= s:
                    hp = s - 1
                    for hh in range(2):
                        h = 2 * hp + hh
                        for ic, icsz in itiles:
                            ot = psO.tile([128, 512], F32, tag="psO",
                                          name="ot")
                            for jt, jsz in jtiles:
                                nc.tensor.matmul(
                                    ot[:icsz, :65],
                                    lhsT=pts[hp][jt][:jsz, hh,
                                                     ic * 128:ic * 128 + icsz],
                                    rhs=v_sb[:jsz, b, jt, h * 65:h * 65 + 65],
                                    start=(jt == 0), stop=(jt == 1))
                            ots[(hh, ic)] = ot
                pop(fill_proj, fill_out)
                if s < HP:
                    for jt, jsz in jtiles:
                        meng.tensor_tensor(pts[s][jt][:jsz], pts[s][jt][:jsz],
                                           mask_sb[:jsz, 2 * jt:2 * jt + 2],
                                           op=ALU.mult)
                if 1 <= s:
                    hp = s - 1
                    for hh in range(2):
                        h = 2 * hp + hh
                        for ic, icsz in itiles:
                            ot = ots.pop((hh, ic))
                            rc = rcp.tile([128, 1], F32, tag="rc", name="rc")
                            nc.vector.reciprocal(rc[:icsz], ot[:icsz, 64:65])
                            nc.vector.tensor_scalar_mul(
                                toks[ic][:icsz, h * 64:h * 64 + 64],
                                ot[:icsz, 0:64], rc[:icsz])
                    pts.pop(hp, None)
                if s >= 2:
                    pop(fill_out, fill_proj)
                else:
                    pop(fill_proj, fill_out)
                pop(fill_proj, fill_out)
                if sprinkle:
                    sprinkle.popleft()()
            return toks

        def make_stage():
            """Allocate one repeat's staged tiles and build its 19 staging
            job emitters (not yet emitted)."""
            tiles = {
                "xT": xTp.tile([128, KT, M_pad], BF16, tag="xT", name="xT"),
                "wqkT": wqp.tile([128, KT, 2 * D], BF16, tag="wqkT",
                                 name="wqkT"),
                "wvT": wvp.tile([128, KT, D], BF16, tag="wvT", name="wvT"),
                "woT": wop.tile([128, KT, D], BF16, tag="woT", name="woT"),
            }
            jobs = {
                "x": pair_jobs(lambda r0, rows: x_flat[r0:r0 + rows, :],
                               M, lambda r0: tiles["xT"][:, :, r0:r0 + 128]),
                "wqk": pair_jobs(lambda r0, rows: wqkv_d[r0:r0 + rows, :],
                                 2 * D,
                                 lambda r0: tiles["wqkT"][:, :, r0:r0 + 128]),
                "wv": pair_jobs(
                    lambda r0, rows: wqkv_d[2 * D + r0:2 * D + r0 + rows, :],
                    D, lambda r0: tiles["wvT"][:, :, r0:r0 + 128]),
                "wout": pair_jobs(lambda r0, rows: wout_d[r0:r0 + rows, :],
                                  D, lambda r0: tiles["woT"][:, :, r0:r0 + 128]),
            }
            return tiles, jobs

        def emit_rep_body(tiles, own_jobs, next_jobs):
            """One repeat's compute. own_jobs is set only for the FIRST
            repeat (cold staging interleaved with the prologue); later
            repeats find their tiles already staged, because each repeat
            sprinkles the NEXT repeat's staging jobs through its cycles
            (the sync SEQ is in-order, so only emission-time interleaving
            can overlap staging DMA with the previous repeat's compute)."""
            cur.clear()
            cur.update(tiles)
            qk_tiles = {}

            def qk_tile(b):
                if b not in qk_tiles:
                    qk_tiles[b] = qkp.tile([128, NT_QK, N], BF16, tag="qk",
                                           name="qk_t")
                return qk_tiles[b]

            qk0 = qkT_item_blobs(0, qk_tile(0))
            v0 = v_item_blobs(0)

            def run_job(j):
                j[0]()
                j[1]()

            if own_jobs is not None:
                # ---- cold prologue: stage and compute hand in hand ----
                xjobs, wqkjobs = own_jobs["x"], own_jobs["wqk"]
                wvjobs, woutjobs = own_jobs["wv"], own_jobs["wout"]
                run_job(wqkjobs[0])
                run_job(xjobs[0])
                run_job(xjobs[1])
                qk0[0]()
                qk0[1]()
                for j in range(1, 6):
                    run_job(wqkjobs[j])  # cols 256j..-: feature tiles 2j,2j+1
                    qk0[2 * j]()
                    qk0[2 * j + 1]()
                run_job(wvjobs[0])
                run_job(wvjobs[1])  # wv rows 0..511: nch0 ready
                v0[0]()
                v0[1]()
                run_job(wvjobs[2])  # nch1 ready
                run_job(xjobs[2])
                v0[2]()
                v0[3]()
                run_job(xjobs[3])
                own_left = [xjobs[4], *woutjobs, *xjobs[5:]]
            else:
                for f in qk0:
                    f()
                for f in v0:
                    f()
                own_left = []
            # sprinkle: leftover cold staging first, then the next repeat's
            # staging jobs. Loads run LAG slots ahead of their convert +
            # transpose phase so no engine queue ever parks on a DMA.
            nxt = []
            if next_jobs is not None:
                nxt = [*next_jobs["wqk"], *next_jobs["x"][:2],
                       *next_jobs["wv"], *next_jobs["x"][2:],
                       *next_jobs["wout"]]
            todo = own_left + nxt
            LAG = 2
            flat = []
            pending = deque()
            for ld, rs in todo:
                flat.append(ld)
                pending.append(rs)
                if len(pending) > LAG:
                    flat.append(pending.popleft())
            flat.extend(pending)
            sprinkle = deque(flat)
            prev_toks = None
            for b in range(bpc):
                fp = deque()
                if b + 1 < bpc:
                    fp.extend(qkT_item_blobs(b + 1, qk_tile(b + 1)))
                    fp.extend(v_item_blobs(b + 1))
                fo = deque()
                if b >= 1:
                    fo.extend(outproj_blobs(b - 1, transpose_attn(prev_toks)))
                prev_toks = emit_attn_weave(b, qk_tile(b), fp, fo, sprinkle)
                for f in fp:
                    f()
                for f in fo:
                    f()
            for f in outproj_blobs(bpc - 1, transpose_attn(prev_toks)):
                f()

        emit_preamble()
        tiles, jobs = make_stage()
        own = jobs
        for _rep in range(repeat):
            if _rep + 1 < repeat:
                ntiles, njobs = make_stage()
            else:
                ntiles, njobs = None, None
            emit_rep_body(tiles, own, njobs)
            tiles, own = ntiles, None

    nc.compile()
    return nc


_NC_CACHE = {}


def _get_nc(bpc=BPC, repeat=1):
    key = (bpc, repeat)
    if key not in _NC_CACHE:
        _NC_CACHE[key] = build_nc(bpc, repeat)
    return _NC_CACHE[key]


def kernel(x, W_qkv, scale, W_out, b_out, _trace=False):
    x = np.ascontiguousarray(np.asarray(x, dtype=np.float32))
    W_qkv = np.ascontiguousarray(np.asarray(W_qkv, dtype=np.float32))
    scale = np.ascontiguousarray(np.asarray(scale, dtype=np.float32))
    W_out = np.ascontiguousarray(np.asarray(W_out, dtype=np.float32))
    b_out = np.ascontiguousarray(np.asarray(b_out, dtype=np.float32))

    nc = _get_nc()
    in_maps = [{
        "x": x[c * BPC:(c + 1) * BPC],
        "w_qkv": W_qkv,
        "scale": scale,
        "w_out": W_out,
        "b_out": b_out,
    } for c in range(NCORES)]
    try:
        res = bass_utils.run_bass_kernel_spmd(
            nc, in_maps, core_ids=list(range(NCORES)), trace=_trace)
    except ModuleNotFoundError:
        # axon NTFF profiling hook unavailable in this container
        res = bass_utils.run_bass_kernel_spmd(
            nc, in_maps, core_ids=list(range(NCORES)), trace=False)
    out = np.concatenate([res.results[c]["out"] for c in range(NCORES)], axis=0)
    if _trace:
        return out, res
    return out


# revision 45
# speedup vs baseline: 1.2355x; 1.0442x over previous
"""Trainium2 Bass kernel for ViT-style LSA attention (sparse_attention).

Reference computation (per batch item):
    qkv = x @ W_qkv.T ; split q,k,v into 12 heads of 64
    dots = (q @ k.T) * scale[h]; diagonal masked to -inf; softmax
    out = (attn @ v) reassembled, then @ W_out.T + b_out

Sharding: data-parallel over batch across 8 NeuronCores (8 items each).

Per-core dataflow (all matmuls bf16 with fp32 PSUM accumulation):
  - x^T and W^T tiles produced on-chip: paired-row DMA loads (fp32) ->
    fp32->bf16 convert (DVE for W_qkv, gpsimd for x/W_out) -> one DMA xbar
    transpose per [128, 768] tile. Staging is INTERLEAVED with compute:
    Tile dependencies follow emission order, so each qkT/V blob is
    emitted right after the staging jobs it needs and waits on nothing
    else (the old stage-everything-first order cost ~50us of PE idle).
  - qk^T = W_qk^T.T @ x^T feature-major, emitted per ITEM (12 feature
    tiles x 197 tokens); the per-head LSA scale is folded into the Q
    tiles during the PSUM->SBUF copy.
  - V natural = x^T.T @ Wv^T token-major, stored per head in 65-wide
    blocks whose 65th column is 1.0 so the PV matmul also produces
    softmax row-sums for free.
  - Attention is SOFTWARE-PIPELINED per item ("weave"): per head pair,
    each j-tile's two S^T matmuls land in a 2-bank PSUM tile, followed
    immediately by one exp (Act) + one mask-multiply covering both
    heads; independent filler blobs (next item's qkT/V projections,
    previous item's output projection, leftover staging jobs) are
    emitted between the S and PV matmuls so the in-order PE queue never
    waits on the Act/DVE softmax chain.
  - out^T[d,i] (+rowsum row 64) per head = V_aug.T @ P^T into a 1-bank
    PSUM tile; normalize with DVE reciprocal + gpsimd partition
    broadcast + DVE multiply into attn^T (feature-major).
  - final = attn^T.T @ W_out^T + b_out (bias via a K=1 ones-row matmul
    in the same PSUM group), PSUM->SBUF copy on DVE, contiguous store.

PSUM budget (8 banks): psS 2x2 + psO 2x1 + psA 2x1.

HW notes (verified the hard way): two matmul accumulation groups may NOT
share a PSUM bank (runtime crash, also with a single start/stop spanning
disjoint ranges); custom-DVE ops (reciprocal_approx_fast) crash at
runtime in this axon environment; DMA cannot read PSUM (bass assert);
issuing a DMA holds the issuing engine's SEQ until the HWDGE queue
grants, so all staging DMAs stay on the sync engine.
"""

from collections import deque
from contextlib import ExitStack

import numpy as np
import ml_dtypes

import concourse.bass as bass
import concourse.bacc as bacc
import concourse.mybir as mybir
import concourse.tile as tile
from concourse import bass_utils, library_config

F32 = mybir.dt.float32
BF16 = mybir.dt.bfloat16
AF = mybir.ActivationFunctionType
ALU = mybir.AluOpType

B, N, D, H, DH = 64, 197, 768, 12, 64
NCORES = 8
BPC = B // NCORES  # batch items per core
KT = D // 128      # 6 contraction tiles of 128
NT_QK = (2 * D) // 128  # 12 feature tiles for q,k

OUT_COPY_DVE = True      # out-projection PSUM->SBUF copy on DVE, not Act
STAGE_CONVERT_POOL = True  # x/W_out staging converts on gpsimd, not DVE
MASK_ON_POOL = False     # diagonal-mask multiply on DVE (Pool is ~3x slower)
N_PAD = 208              # 197 tokens padded to a multiple of 16 for the xbar


def build_nc(bpc=BPC, repeat=1):
    """Build the kernel. repeat>1 emits the whole body N times back-to-back
    (used only for timing: differencing two repeat counts cancels the fixed
    PJRT dispatch + host<->device transfer overhead)."""
    M = bpc * N  # tokens per core

    nc = bacc.Bacc("TRN2", target_bir_lowering=False, debug=False,
                   num_devices=NCORES)
    x_d = nc.dram_tensor("x", [bpc, N, D], F32, kind="ExternalInput")
    wqkv_d = nc.dram_tensor("w_qkv", [3 * D, D], F32, kind="ExternalInput")
    scale_d = nc.dram_tensor("scale", [H], F32, kind="ExternalInput")
    wout_d = nc.dram_tensor("w_out", [D, D], F32, kind="ExternalInput")
    bout_d = nc.dram_tensor("b_out", [D], F32, kind="ExternalInput")
    out_d = nc.dram_tensor("out", [bpc, N, D], F32, kind="ExternalOutput")

    # Multiplicative diagonal mask for P^T tiles, laid out to match the
    # S tiles: mask4[p, 2*jt+hh, i] = 0 iff i == jt*128 + p.
    mask_np = np.ones((128, 4, N), dtype=ml_dtypes.bfloat16)
    for jt in range(2):
        for p in range(128):
            i = jt * 128 + p
            if i < N:
                mask_np[p, 2 * jt, i] = 0
                mask_np[p, 2 * jt + 1, i] = 0
    mask_d = nc.inline_tensor(mask_np, name="maskc")
    ones_d = nc.inline_tensor(np.ones((1, 128), dtype=ml_dtypes.bfloat16),
                              name="onesc")

    x_flat = x_d[:, :, :].flatten_outer_dims()  # [M, D]
    jtiles = [(0, 128), (1, N - 128)]

    with tile.TileContext(nc) as tc, ExitStack() as es:
        res = es.enter_context(tc.tile_pool(name="res", bufs=1))

        nc.gpsimd.load_library(library_config.attn)

        # ---- resident tiles (allocated once, written by each repeat) ----
        mask_sb = res.tile([128, 4, N], BF16, name="mask_sb")
        ones_bf = res.tile([1, 128], BF16, name="ones_bf")
        scale_row = res.tile([1, H], F32, name="scale_row")
        scale_bc = res.tile([128, H], F32, name="scale_bc")
        scale_bc2 = res.tile([128, KT, 1], F32, name="scale_bc2")
        brow = res.tile([1, D], F32, name="brow")
        bout_bf = res.tile([1, D], BF16, name="bout_bf")
        # token dim padded to 128 so the xbar transpose always moves full
        # [128, 128] tiles (row count must be a multiple of 16); the padding
        # is zero-filled and never read by any matmul.
        M_pad = ((M + 127) // 128) * 128
        v_sb = res.tile([128, bpc, 2, H * 65], BF16, name="v_sb")

        # ---- pools ----
        # The staged-transpose targets are double-buffered (bufs=2) so each
        # repeat's staging DMA overlaps the previous repeat's compute tail;
        # qk^T is pooled PER ITEM (alive ~2 cycles) instead of resident.
        xTp = es.enter_context(tc.tile_pool(name="xTp", bufs=2))
        wqp = es.enter_context(tc.tile_pool(name="wqp", bufs=2))
        wvp = es.enter_context(tc.tile_pool(name="wvp", bufs=2))
        wop = es.enter_context(tc.tile_pool(name="wop", bufs=2))
        qkp = es.enter_context(tc.tile_pool(name="qkp", bufs=3))
        stg = es.enter_context(tc.tile_pool(name="stg", bufs=3))
        stgb = es.enter_context(tc.tile_pool(name="stgb", bufs=2))
        # PSUM pools: 8 banks total = psS 1x4 + psO 2x1 + psA 2x1.
        psA = es.enter_context(tc.tile_pool(name="psA", bufs=2, space="PSUM"))
        psS = es.enter_context(tc.tile_pool(name="psS", bufs=1, space="PSUM"))
        psO = es.enter_context(tc.tile_pool(name="psO", bufs=2, space="PSUM"))
        ptp = es.enter_context(tc.tile_pool(name="ptp", bufs=4))
        rcp = es.enter_context(tc.tile_pool(name="rcp", bufs=4))
        akp = es.enter_context(tc.tile_pool(name="akp", bufs=4))
        atp = es.enter_context(tc.tile_pool(name="atp", bufs=2))
        osp = es.enter_context(tc.tile_pool(name="osp", bufs=2))

        def stage_load(src_ap, nrows, n2):
            """Phase 1 of a staging job: paired-row DMA load (fp32)."""
            t_f = stg.tile([128, 2, D], F32, tag="stg", name="t_f")
            if nrows < n2 * 128:
                nc.vector.memset(t_f, 0.0)
            if nrows > 128:
                nc.sync.dma_start(
                    t_f[:, :2], src_ap.rearrange("(t p) f -> p t f", p=128))
            else:
                nc.sync.dma_start(t_f[:nrows, 0], src_ap)
            return t_f

        def stage_rest(t_f, dsts, n2):
            """Phase 2: fp32->bf16 convert (gpsimd) + one xbar transpose
            per [128, 768] row-tile. Emitted a couple of pipeline slots
            after phase 1 so the convert never reaches the Pool queue head
            before its DMA load has finished (every engine stream is
            in-order: a waiting op stalls everything behind it)."""
            t_b = stgb.tile([128, 2, D], BF16, tag="stgb", name="t_b")
            nc.gpsimd.tensor_copy(t_b[:, :n2], t_f[:, :n2])
            for t, dst in enumerate(dsts):
                if dst is None:
                    continue
                # one xbar transpose per [128, 768] tile: 3D dst gets row
                # kt*128+p at [p, kt, m] (sim-verified)
                nc.sync.dma_start_transpose(dst, t_b[:, t])

        def pair_jobs(src_rows, total_rows, dst_fn):
            """Two-phase (load, rest) emitter pairs, 256 rows apiece."""
            jobs = []
            r0 = 0
            while r0 < total_rows:
                rows = min(256, total_rows - r0)
                if rows < 256:
                    rows = min(128, rows)  # singles for the tail
                d0 = dst_fn(r0)
                d1 = dst_fn(r0 + 128) if rows > 128 else None
                box = {}

                def load(box=box, s=src_rows(r0, rows), n=rows,
                         n2=(2 if rows > 128 else 1)):
                    box["t"] = stage_load(s, n, n2)

                def rest(box=box, d=(d0, d1), n2=(2 if rows > 128 else 1)):
                    stage_rest(box.pop("t"), d, n2)

                jobs.append((load, rest))
                r0 += rows
            return jobs

        def emit_preamble():
            """Constant setup, emitted ONCE (not per repeat): these tiles
            hold input-derived constants that no repeat overwrites. A
            per-repeat re-DMA would sem-wait on the previous repeat's
            readers while holding the sync SEQ, stalling the whole DMA
            queue at each repeat seam."""
            nc.sync.dma_start(mask_sb, mask_d[:, :, :])
            nc.sync.dma_start(ones_bf, ones_d[:, :])
            nc.sync.dma_start(scale_row, scale_d[None, :])
            nc.gpsimd.partition_broadcast(scale_bc, scale_row)
            for nt in range(KT):
                for hh in range(2):
                    nc.vector.tensor_copy(
                        scale_bc2[hh * 64:(hh + 1) * 64, nt],
                        scale_bc[hh * 64:(hh + 1) * 64,
                                 2 * nt + hh:2 * nt + hh + 1])
            nc.sync.dma_start(brow, bout_d[None, :])
            nc.vector.tensor_copy(bout_bf, brow)
            nc.vector.memset(
                v_sb.rearrange("p b j (h e) -> p b j h e",
                               e=65)[:, :, :, :, 64:65], 1.0)

        cur = {}  # per-repeat staged tiles (rotated for cross-rep overlap)

        def qkT_item_blobs(b, qk_t):
            """12 filler blobs: qk^T feature tiles for item b's tokens."""
            t0 = b * N
            blobs = []
            for nt in range(NT_QK):
                def blob(nt=nt):
                    ps = psA.tile([128, 512], F32, tag="psA", name="ps_qk")
                    for kt in range(KT):
                        nc.tensor.matmul(
                            ps[:, :N],
                            lhsT=cur["wqkT"][:, kt, nt * 128:(nt + 1) * 128],
                            rhs=cur["xT"][:, kt, t0:t0 + N],
                            start=(kt == 0), stop=(kt == KT - 1))
                    if nt < KT:  # Q tiles: fold in the per-head LSA scale
                        # on DVE (tensor_scalar with per-partition scale):
                        # Act is the more loaded engine
                        nc.vector.tensor_scalar_mul(qk_t[:, nt, :],
                                                    ps[:, :N],
                                                    scale_bc2[:, nt])
                    else:
                        nc.scalar.copy(qk_t[:, nt, :], ps[:, :N])
                blobs.append(blob)
            return blobs

        def v_item_blobs(b):
            """4 filler blobs: token-major V (+ones col) for item b,
            ordered (jt0,nch0), (jt1,nch0), (jt0,nch1), (jt1,nch1)."""
            blobs = []
            for nch in range(2):
                for jt, jsz in jtiles:
                    def blob(jt=jt, jsz=jsz, nch=nch):
                        ps = psA.tile([128, 512], F32, tag="psA", name="ps_v")
                        for kt in range(KT):
                            nc.tensor.matmul(
                                ps[:jsz, :384],
                                lhsT=cur["xT"][:, kt,
                                               b * N + jt * 128:
                                               b * N + jt * 128 + jsz],
                                rhs=cur["wvT"][:, kt,
                                               nch * 384:(nch + 1) * 384],
                                start=(kt == 0), stop=(kt == KT - 1))
                        dst = v_sb[:jsz, b, jt].rearrange(
                            "p (h e) -> p h e",
                            e=65)[:, nch * 6:(nch + 1) * 6, 0:64]
                        nc.scalar.copy(
                            dst,
                            ps[:jsz, :384].rearrange("p (h e) -> p h e", e=64))
                    blobs.append(blob)
            return blobs

        def outproj_blobs(b, attnT):
            """4 filler blobs: output projection halves for item b."""
            blobs = []
            for jt, jsz in jtiles:
                cell = []

                def half(jt, jsz, nch, cell):
                    ps = psA.tile([128, 512], F32, tag="psA", name="ps_o")
                    for ft in range(KT):
                        nc.tensor.matmul(
                            ps[:jsz, :384],
                            lhsT=attnT[:, ft, jt * 128:jt * 128 + jsz],
                            rhs=cur["woT"][:, ft, nch * 384:(nch + 1) * 384],
                            start=(ft == 0), stop=False)
                    nc.tensor.matmul(
                        ps[:jsz, :384], lhsT=ones_bf[0:1, :jsz],
                        rhs=bout_bf[0:1, nch * 384:(nch + 1) * 384],
                        start=False, stop=True)
                    if OUT_COPY_DVE:
                        nc.vector.tensor_copy(
                            cell[0][:jsz, nch * 384:(nch + 1) * 384],
                            ps[:jsz, :384])
                    else:
                        nc.scalar.copy(
                            cell[0][:jsz, nch * 384:(nch + 1) * 384],
                            ps[:jsz, :384])

                def blob0(jt=jt, jsz=jsz, cell=cell):
                    cell.append(osp.tile([128, D], F32, tag="osb", name="osb"))
                    half(jt, jsz, 0, cell)

                def blob1(jt=jt, jsz=jsz, cell=cell):
                    half(jt, jsz, 1, cell)
                    # one fully-contiguous [jsz, 768] store per (item, j-tile)
                    nc.sync.dma_start(out_d[b, jt * 128:jt * 128 + jsz, :],
                                      cell[0][:jsz])
                blobs += [blob0, blob1]
            return blobs

        def transpose_attn(toks):
            """One xbar transpose pass: token-major attn [i, f] (two row
            tiles, the second padded to 80 rows) -> feature-major attn^T
            [f-part, kt, i] for the output projection. Columns 197..207
            receive garbage from the pad rows and are never read."""
            attnT = atp.tile([128, KT, N_PAD], BF16, tag="attnT",
                             name="attnT")
            # issued from the Act DGE queue (only SP/Act can drive HWDGE):
            # skips the load/store-laden sync queue so the out-projection
            # isn't stuck behind staging transfers.
            nc.scalar.dma_start_transpose(attnT[:, :, 0:128], toks[0])
            nc.scalar.dma_start_transpose(attnT[:, :, 128:N_PAD],
                                          toks[1][0:80])
            return attnT

        def emit_attn_weave(b, qk_t, fill_proj, fill_out, sprinkle):
            """Attention for item b, software-pipelined: filler blobs are
            popped between the S and PV matmuls of each head pair so the
            in-order PE stream never waits on the Act/DVE softmax chain.
            `sprinkle` holds staging jobs (DMA-side) to emit along the way."""

            def pop(dq1, dq2):
                if dq1:
                    dq1.popleft()()
                elif dq2:
                    dq2.popleft()()

            meng = nc.gpsimd if MASK_ON_POOL else nc.vector
            HP = H // 2
            itiles = [(0, 128), (1, N - 128)]
            pts, ots = {}, {}
            # TOKEN-MAJOR PV: out[i, d] = P^T.T @ V_aug puts query tokens on
            # the PSUM partitions, so the softmax row-sum (65th column) is a
            # PER-PARTITION scalar: normalize is a [jsz,1] reciprocal + a
            # per-partition tensor_scalar multiply — no partition broadcast,
            # no wide multiplies. The chain is stage-lagged (every engine
            # stream is in-order, so a sem-wait at the queue head stalls all
            # later ops): S/exp @ s, mask @ s (after a filler), PV @ s+1,
            # recip+mul @ s+1 (after a filler). attn lands token-major in
            # SBUF; one xbar transpose per item (next cycle) rebuilds the
            # feature-major attn^T that the output projection consumes.
            toks = [akp.tile([128, D], BF16, tag="tok", name="tok")
                    for _ in range(2)]
            for s in range(HP + 1):
                if s < HP:
                    # All four of the head pair's S^T matmuls (2 j-tiles x
                    # 2 heads) into one 4-bank tile, then ONE exp covering
                    # every slot (rows >= jsz of the j-tile-1 slots hold
                    # garbage no consumer reads).
                    st4 = psS.tile([128, 4, N], F32, tag="psS", name="st4",
                                   padded_shape=[128, 4, 512])
                    for jt, jsz in jtiles:
                        for hh in range(2):
                            pb = hh * 64
                            qa = qk_t[pb:pb + 64, s, :]
                            ka = qk_t[pb:pb + 64, 6 + s, :]
                            nc.tensor.matmul(
                                st4[:jsz, 2 * jt + hh],
                                lhsT=ka[:, jt * 128:jt * 128 + jsz],
                                rhs=qa, start=True, stop=True)
                    pt4 = ptp.tile([128, 4, N], BF16, tag="pt", name="pt4")
                    nc.scalar.activation(pt4, st4, AF.Exp)
                    pts[s] = pt4
                def pv_norm(hp, hh):
                    # PV + normalize for one head, both i-chunks, with the
                    # normalize (tiny DVE recip + per-partition multiply)
                    # emitted immediately so each 1-bank psO tile frees
                    # within half a stage (only 2 bufs exist).
                    h = 2 * hp + hh
                    for ic, icsz in itiles:
                        ot = psO.tile([128, 512], F32, tag="psO", name="ot")
                        for jt, jsz in jtiles:
                            nc.tensor.matmul(
                                ot[:icsz, :65],
                                lhsT=pts[hp][:jsz, 2 * jt + hh,
                                             ic * 128:ic * 128 + icsz],
                                rhs=v_sb[:jsz, b, jt, h * 65:h * 65 + 65],
                                start=(jt == 0), stop=(jt == 1))
                        rc = rcp.tile([128, 1], F32, tag="rc", name="rc")
                        nc.vector.reciprocal(rc[:icsz], ot[:icsz, 64:65])
                        nc.vector.tensor_scalar_mul(
                            toks[ic][:icsz, h * 64:h * 64 + 64],
                            ot[:icsz, 0:64], rc[:icsz])

                if 1 <= s:
                    pv_norm(s - 1, 0)
                pop(fill_proj, fill_out)
                if s < HP:
                    # ONE mask multiply over all 4 slots (layout matches)
                    meng.tensor_tensor(pts[s], pts[s], mask_sb, op=ALU.mult)
                if 1 <= s:
                    pv_norm(s - 1, 1)
                    pts.pop(s - 1, None)
                if s >= 2:
                    pop(fill_out, fill_proj)
                else:
                    pop(fill_proj, fill_out)
                pop(fill_proj, fill_out)
                if sprinkle:
                    sprinkle.popleft()()
            return toks

        def make_stage():
            """Allocate one repeat's staged tiles and build its 19 staging
            job emitters (not yet emitted)."""
            tiles = {
                "xT": xTp.tile([128, KT, M_pad], BF16, tag="xT", name="xT"),
                "wqkT": wqp.tile([128, KT, 2 * D], BF16, tag="wqkT",
                                 name="wqkT"),
                "wvT": wvp.tile([128, KT, D], BF16, tag="wvT", name="wvT"),
                "woT": wop.tile([128, KT, D], BF16, tag="woT", name="woT"),
            }
            jobs = {
                "x": pair_jobs(lambda r0, rows: x_flat[r0:r0 + rows, :],
                               M, lambda r0: tiles["xT"][:, :, r0:r0 + 128]),
                "wqk": pair_jobs(lambda r0, rows: wqkv_d[r0:r0 + rows, :],
                                 2 * D,
                                 lambda r0: tiles["wqkT"][:, :, r0:r0 + 128]),
                "wv": pair_jobs(
                    lambda r0, rows: wqkv_d[2 * D + r0:2 * D + r0 + rows, :],
                    D, lambda r0: tiles["wvT"][:, :, r0:r0 + 128]),
                "wout": pair_jobs(lambda r0, rows: wout_d[r0:r0 + rows, :],
                                  D, lambda r0: tiles["woT"][:, :, r0:r0 + 128]),
            }
            return tiles, jobs

        def emit_rep_body(tiles, own_jobs, next_jobs):
            """One repeat's compute. own_jobs is set only for the FIRST
            repeat (cold staging interleaved with the prologue); later
            repeats find their tiles already staged, because each repeat
            sprinkles the NEXT repeat's staging jobs through its cycles
            (the sync SEQ is in-order, so only emission-time interleaving
            can overlap staging DMA with the previous repeat's compute)."""
            cur.clear()
            cur.update(tiles)
            qk_tiles = {}

            def qk_tile(b):
                if b not in qk_tiles:
                    qk_tiles[b] = qkp.tile([128, NT_QK, N], BF16, tag="qk",
                                           name="qk_t")
                return qk_tiles[b]

            qk0 = qkT_item_blobs(0, qk_tile(0))
            v0 = v_item_blobs(0)

            def run_job(j):
                j[0]()
                j[1]()

            if own_jobs is not None:
                # ---- cold prologue: stage and compute hand in hand ----
                xjobs, wqkjobs = own_jobs["x"], own_jobs["wqk"]
                wvjobs, woutjobs = own_jobs["wv"], own_jobs["wout"]
                run_job(wqkjobs[0])
                run_job(xjobs[0])
                run_job(xjobs[1])
                qk0[0]()
                qk0[1]()
                for j in range(1, 6):
                    run_job(wqkjobs[j])  # cols 256j..-: feature tiles 2j,2j+1
                    qk0[2 * j]()
                    qk0[2 * j + 1]()
                run_job(wvjobs[0])
                run_job(wvjobs[1])  # wv rows 0..511: nch0 ready
                v0[0]()
                v0[1]()
                run_job(wvjobs[2])  # nch1 ready
                run_job(xjobs[2])
                v0[2]()
                v0[3]()
                run_job(xjobs[3])
                own_left = [xjobs[4], *woutjobs, *xjobs[5:]]
            else:
                for f in qk0:
                    f()
                for f in v0:
                    f()
                own_left = []
            # sprinkle: leftover cold staging first, then the next repeat's
            # staging jobs. Loads run LAG slots ahead of their convert +
            # transpose phase so no engine queue ever parks on a DMA.
            nxt = []
            if next_jobs is not None:
                nxt = [*next_jobs["wqk"], *next_jobs["x"][:2],
                       *next_jobs["wv"], *next_jobs["x"][2:],
                       *next_jobs["wout"]]
            todo = own_left + nxt
            LAG = 2
            flat = []
            pending = deque()
            for ld, rs in todo:
                flat.append(ld)
                pending.append(rs)
                if len(pending) > LAG:
                    flat.append(pending.popleft())
            flat.extend(pending)
            sprinkle = deque(flat)
            prev_toks = None
            for b in range(bpc):
                fp = deque()
                if b + 1 < bpc:
                    fp.extend(qkT_item_blobs(b + 1, qk_tile(b + 1)))
                    fp.extend(v_item_blobs(b + 1))
                fo = deque()
                if b >= 1:
                    fo.extend(outproj_blobs(b - 1, transpose_attn(prev_toks)))
                prev_toks = emit_attn_weave(b, qk_tile(b), fp, fo, sprinkle)
                for f in fp:
                    f()
                for f in fo:
                    f()
            for f in outproj_blobs(bpc - 1, transpose_attn(prev_toks)):
                f()

        emit_preamble()
        tiles, jobs = make_stage()
        own = jobs
        for _rep in range(repeat):
            if _rep + 1 < repeat:
                ntiles, njobs = make_stage()
            else:
                ntiles, njobs = None, None
            emit_rep_body(tiles, own, njobs)
            tiles, own = ntiles, None

    nc.compile()
    return nc


_NC_CACHE = {}


def _get_nc(bpc=BPC, repeat=1):
    key = (bpc, repeat)
    if key not in _NC_CACHE:
        _NC_CACHE[key] = build_nc(bpc, repeat)
    return _NC_CACHE[key]


def kernel(x, W_qkv, scale, W_out, b_out, _trace=False):
    x = np.ascontiguousarray(np.asarray(x, dtype=np.float32))
    W_qkv = np.ascontiguousarray(np.asarray(W_qkv, dtype=np.float32))
    scale = np.ascontiguousarray(np.asarray(scale, dtype=np.float32))
    W_out = np.ascontiguousarray(np.asarray(W_out, dtype=np.float32))
    b_out = np.ascontiguousarray(np.asarray(b_out, dtype=np.float32))

    nc = _get_nc()
    in_maps = [{
        "x": x[c * BPC:(c + 1) * BPC],
        "w_qkv": W_qkv,
        "scale": scale,
        "w_out": W_out,
        "b_out": b_out,
    } for c in range(NCORES)]
    try:
        res = bass_utils.run_bass_kernel_spmd(
            nc, in_maps, core_ids=list(range(NCORES)), trace=_trace)
    except ModuleNotFoundError:
        # axon NTFF profiling hook unavailable in this container
        res = bass_utils.run_bass_kernel_spmd(
            nc, in_maps, core_ids=list(range(NCORES)), trace=False)
    out = np.concatenate([res.results[c]["out"] for c in range(NCORES)], axis=0)
    if _trace:
        return out, res
    return out


# revision 66
# speedup vs baseline: 1.4070x; 1.1388x over previous
"""Trainium2 Bass kernel for ViT-style LSA attention (sparse_attention).

Reference computation (per batch item):
    qkv = x @ W_qkv.T ; split q,k,v into 12 heads of 64
    dots = (q @ k.T) * scale[h]; diagonal masked to -inf; softmax
    out = (attn @ v) reassembled, then @ W_out.T + b_out

Sharding: data-parallel over batch across 8 NeuronCores (8 items each).

Per-core dataflow (all matmuls bf16 with fp32 PSUM accumulation):
  - x^T and W^T tiles produced on-chip: paired-row DMA loads (fp32) ->
    fp32->bf16 convert (DVE for W_qkv, gpsimd for x/W_out) -> one DMA xbar
    transpose per [128, 768] tile. Staging is INTERLEAVED with compute:
    Tile dependencies follow emission order, so each qkT/V blob is
    emitted right after the staging jobs it needs and waits on nothing
    else (the old stage-everything-first order cost ~50us of PE idle).
  - qk^T = W_qk^T.T @ x^T feature-major, emitted per ITEM (12 feature
    tiles x 197 tokens); the per-head LSA scale is folded into the Q
    tiles during the PSUM->SBUF copy.
  - V natural = x^T.T @ Wv^T token-major, stored per head in 65-wide
    blocks whose 65th column is 1.0 so the PV matmul also produces
    softmax row-sums for free.
  - Attention is SOFTWARE-PIPELINED per item ("weave"): per head pair,
    all four S^T matmuls (2 j-tiles x 2 heads) land in one 4-bank PSUM
    tile, followed by ONE exp (Act) + ONE mask-multiply (DVE) covering
    every slot; independent filler blobs (next item's qkT/V projections,
    previous item's output projection, staged loads for the next repeat)
    are emitted between the S and PV matmuls so the in-order PE queue
    never waits on the softmax chain.
  - PV is TOKEN-MAJOR: out[i, 65] = P^T.T @ V_aug puts query tokens on
    the PSUM partitions, so the softmax row-sum (column 64) is a
    per-partition scalar and normalization collapses to a [jsz,1] DVE
    reciprocal + per-partition tensor_scalar multiply — no partition
    broadcast, no wide multiplies. One xbar transpose per item (on the
    Act DGE queue) rebuilds feature-major attn^T for the projection.
  - final = attn^T.T @ W_out^T; the bias add is fused into the DVE
    PSUM->SBUF copy against a preamble-broadcast bias tile (no K=1
    bias matmuls on the PE); out-projection of item b runs two cycles
    after its attention so the attn^T transpose has a full cycle of
    DMA-queue slack. Contiguous [jsz, 768] stores.

PSUM budget (8 banks): psS 1x4 + psO 2x1 + psA 2x1.

HW notes (verified the hard way): two matmul accumulation groups may NOT
share a PSUM bank (runtime crash, also with a single start/stop spanning
disjoint ranges); custom-DVE ops (reciprocal_approx_fast) crash at
runtime in this axon environment; DMA cannot read PSUM (bass assert);
issuing a DMA holds the issuing engine's SEQ until the HWDGE queue
grants, so all staging DMAs stay on the sync engine.
"""

from collections import deque
from contextlib import ExitStack

import numpy as np
import ml_dtypes

import concourse.bass as bass
import concourse.bacc as bacc
import concourse.mybir as mybir
import concourse.tile as tile
from concourse import bass_utils, library_config

F32 = mybir.dt.float32
BF16 = mybir.dt.bfloat16
AF = mybir.ActivationFunctionType
ALU = mybir.AluOpType

B, N, D, H, DH = 64, 197, 768, 12, 64
NCORES = 8
BPC = B // NCORES  # batch items per core
KT = D // 128      # 6 contraction tiles of 128
NT_QK = (2 * D) // 128  # 12 feature tiles for q,k

OUT_COPY_DVE = True      # out-projection PSUM->SBUF copy on DVE, not Act
STAGE_CONVERT_POOL = True  # x/W_out staging converts on gpsimd, not DVE
MASK_ON_POOL = False     # diagonal-mask multiply on DVE (Pool is ~3x slower)
N_PAD = 208              # 197 tokens padded to a multiple of 16 for the xbar


def build_nc(bpc=BPC, repeat=1):
    """Build the kernel. repeat>1 emits the whole body N times back-to-back
    (used only for timing: differencing two repeat counts cancels the fixed
    PJRT dispatch + host<->device transfer overhead)."""
    M = bpc * N  # tokens per core

    nc = bacc.Bacc("TRN2", target_bir_lowering=False, debug=False,
                   num_devices=NCORES)
    x_d = nc.dram_tensor("x", [bpc, N, D], F32, kind="ExternalInput")
    wqkv_d = nc.dram_tensor("w_qkv", [3 * D, D], F32, kind="ExternalInput")
    scale_d = nc.dram_tensor("scale", [H], F32, kind="ExternalInput")
    wout_d = nc.dram_tensor("w_out", [D, D], F32, kind="ExternalInput")
    bout_d = nc.dram_tensor("b_out", [D], F32, kind="ExternalInput")
    out_d = nc.dram_tensor("out", [bpc, N, D], F32, kind="ExternalOutput")

    # Multiplicative diagonal mask for P^T tiles, laid out to match the
    # S tiles: mask4[p, 2*jt+hh, i] = 0 iff i == jt*128 + p.
    mask_np = np.ones((128, 4, N), dtype=ml_dtypes.bfloat16)
    for jt in range(2):
        for p in range(128):
            i = jt * 128 + p
            if i < N:
                mask_np[p, 2 * jt, i] = 0
                mask_np[p, 2 * jt + 1, i] = 0
    mask_d = nc.inline_tensor(mask_np, name="maskc")

    x_flat = x_d[:, :, :].flatten_outer_dims()  # [M, D]
    jtiles = [(0, 128), (1, N - 128)]

    with tile.TileContext(nc) as tc, ExitStack() as es:
        res = es.enter_context(tc.tile_pool(name="res", bufs=1))

        nc.gpsimd.load_library(library_config.attn)

        # ---- resident tiles (allocated once, written by each repeat) ----
        mask_sb = res.tile([128, 4, N], BF16, name="mask_sb")
        scale_row = res.tile([1, H], F32, name="scale_row")
        scale_bc = res.tile([128, H], F32, name="scale_bc")
        scale_bc2 = res.tile([128, KT, 1], F32, name="scale_bc2")
        brow = res.tile([1, D], F32, name="brow")
        bias_bc = res.tile([128, D], F32, name="bias_bc")
        # token dim padded to 128 so the xbar transpose always moves full
        # [128, 128] tiles (row count must be a multiple of 16); the padding
        # is zero-filled and never read by any matmul.
        M_pad = ((M + 127) // 128) * 128
        v_sb = res.tile([128, bpc, 2, H * 65], BF16, name="v_sb")

        # ---- pools ----
        # The staged-transpose targets are double-buffered (bufs=2) so each
        # repeat's staging DMA overlaps the previous repeat's compute tail;
        # qk^T is pooled PER ITEM (alive ~2 cycles) instead of resident.
        xTp = es.enter_context(tc.tile_pool(name="xTp", bufs=2))
        wqp = es.enter_context(tc.tile_pool(name="wqp", bufs=2))
        wvp = es.enter_context(tc.tile_pool(name="wvp", bufs=2))
        wop = es.enter_context(tc.tile_pool(name="wop", bufs=2))
        qkp = es.enter_context(tc.tile_pool(name="qkp", bufs=3))
        stg = es.enter_context(tc.tile_pool(name="stg", bufs=3))
        stgb = es.enter_context(tc.tile_pool(name="stgb", bufs=2))
        # PSUM pools: 8 banks total = psS 1x4 + psO 2x1 + psA 2x1.
        psA = es.enter_context(tc.tile_pool(name="psA", bufs=2, space="PSUM"))
        psS = es.enter_context(tc.tile_pool(name="psS", bufs=1, space="PSUM"))
        psO = es.enter_context(tc.tile_pool(name="psO", bufs=2, space="PSUM"))
        ptp = es.enter_context(tc.tile_pool(name="ptp", bufs=4))
        rcp = es.enter_context(tc.tile_pool(name="rcp", bufs=4))
        akp = es.enter_context(tc.tile_pool(name="akp", bufs=4))
        atp = es.enter_context(tc.tile_pool(name="atp", bufs=3))
        osp = es.enter_context(tc.tile_pool(name="osp", bufs=2))

        def stage_load(src_ap, nrows, n2):
            """Phase 1 of a staging job: paired-row DMA load (fp32)."""
            t_f = stg.tile([128, 2, D], F32, tag="stg", name="t_f")
            if nrows < n2 * 128:
                nc.vector.memset(t_f, 0.0)
            if nrows > 128:
                nc.sync.dma_start(
                    t_f[:, :2], src_ap.rearrange("(t p) f -> p t f", p=128))
            else:
                nc.sync.dma_start(t_f[:nrows, 0], src_ap)
            return t_f

        def stage_rest(t_f, dsts, n2):
            """Phase 2: fp32->bf16 convert (gpsimd) + one xbar transpose
            per [128, 768] row-tile. Emitted a couple of pipeline slots
            after phase 1 so the convert never reaches the Pool queue head
            before its DMA load has finished (every engine stream is
            in-order: a waiting op stalls everything behind it)."""
            t_b = stgb.tile([128, 2, D], BF16, tag="stgb", name="t_b")
            nc.gpsimd.tensor_copy(t_b[:, :n2], t_f[:, :n2])
            for t, dst in enumerate(dsts):
                if dst is None:
                    continue
                # one xbar transpose per [128, 768] tile: 3D dst gets row
                # kt*128+p at [p, kt, m] (sim-verified)
                nc.sync.dma_start_transpose(dst, t_b[:, t])

        def pair_jobs(src_rows, total_rows, dst_fn):
            """Two-phase (load, rest) emitter pairs, 256 rows apiece."""
            jobs = []
            r0 = 0
            while r0 < total_rows:
                rows = min(256, total_rows - r0)
                if rows < 256:
                    rows = min(128, rows)  # singles for the tail
                d0 = dst_fn(r0)
                d1 = dst_fn(r0 + 128) if rows > 128 else None
                box = {}

                def load(box=box, s=src_rows(r0, rows), n=rows,
                         n2=(2 if rows > 128 else 1)):
                    box["t"] = stage_load(s, n, n2)

                def rest(box=box, d=(d0, d1), n2=(2 if rows > 128 else 1)):
                    stage_rest(box.pop("t"), d, n2)

                jobs.append((load, rest))
                r0 += rows
            return jobs

        def emit_preamble():
            """Constant setup, emitted ONCE (not per repeat): these tiles
            hold input-derived constants that no repeat overwrites. A
            per-repeat re-DMA would sem-wait on the previous repeat's
            readers while holding the sync SEQ, stalling the whole DMA
            queue at each repeat seam."""
            nc.sync.dma_start(mask_sb, mask_d[:, :, :])
            nc.sync.dma_start(scale_row, scale_d[None, :])
            nc.gpsimd.partition_broadcast(scale_bc, scale_row)
            for nt in range(KT):
                for hh in range(2):
                    nc.vector.tensor_copy(
                        scale_bc2[hh * 64:(hh + 1) * 64, nt],
                        scale_bc[hh * 64:(hh + 1) * 64,
                                 2 * nt + hh:2 * nt + hh + 1])
            nc.sync.dma_start(brow, bout_d[None, :])
            # bias broadcast to all partitions once: the out-projection's
            # PSUM->SBUF copy adds it as a plain tensor_tensor, replacing
            # the per-tile K=1 ones-row bias matmuls on the PE.
            nc.gpsimd.partition_broadcast(bias_bc, brow)
            nc.vector.memset(
                v_sb.rearrange("p b j (h e) -> p b j h e",
                               e=65)[:, :, :, :, 64:65], 1.0)

        cur = {}  # per-repeat staged tiles (rotated for cross-rep overlap)

        def qkT_item_blobs(b, qk_t):
            """12 filler blobs: qk^T feature tiles for item b's tokens."""
            t0 = b * N
            blobs = []
            for nt in range(NT_QK):
                def blob(nt=nt):
                    ps = psA.tile([128, 512], F32, tag="psA", name="ps_qk")
                    for kt in range(KT):
                        nc.tensor.matmul(
                            ps[:, :N],
                            lhsT=cur["wqkT"][:, kt, nt * 128:(nt + 1) * 128],
                            rhs=cur["xT"][:, kt, t0:t0 + N],
                            start=(kt == 0), stop=(kt == KT - 1))
                    if nt < KT:  # Q tiles: fold in the per-head LSA scale
                        # on DVE (tensor_scalar with per-partition scale):
                        # Act is the more loaded engine
                        nc.vector.tensor_scalar_mul(qk_t[:, nt, :],
                                                    ps[:, :N],
                                                    scale_bc2[:, nt])
                    else:
                        nc.scalar.copy(qk_t[:, nt, :], ps[:, :N])
                blobs.append(blob)
            return blobs

        def v_item_blobs(b):
            """4 filler blobs: token-major V (+ones col) for item b,
            ordered (jt0,nch0), (jt1,nch0), (jt0,nch1), (jt1,nch1)."""
            blobs = []
            for nch in range(2):
                for jt, jsz in jtiles:
                    def blob(jt=jt, jsz=jsz, nch=nch):
                        ps = psA.tile([128, 512], F32, tag="psA", name="ps_v")
                        for kt in range(KT):
                            nc.tensor.matmul(
                                ps[:jsz, :384],
                                lhsT=cur["xT"][:, kt,
                                               b * N + jt * 128:
                                               b * N + jt * 128 + jsz],
                                rhs=cur["wvT"][:, kt,
                                               nch * 384:(nch + 1) * 384],
                                start=(kt == 0), stop=(kt == KT - 1))
                        dst = v_sb[:jsz, b, jt].rearrange(
                            "p (h e) -> p h e",
                            e=65)[:, nch * 6:(nch + 1) * 6, 0:64]
                        nc.scalar.copy(
                            dst,
                            ps[:jsz, :384].rearrange("p (h e) -> p h e", e=64))
                    blobs.append(blob)
            return blobs

        def outproj_blobs(b, attnT, woT, seam=False):
            """4 filler blobs: output projection halves for item b. woT is
            captured at build time because the tail items' blobs are
            carried into the NEXT repeat's prologue (whose cur[] differs).
            seam=True routes the PSUM->SBUF copy to Act + the bias add to
            Pool: at a repeat seam the in-order DVE queue still holds the
            last item's whole softmax chain, and a DVE copy queued behind
            it would gate the next repeat's psA rotation for ~25us."""
            blobs = []
            for jt, jsz in jtiles:
                cell = []

                def half(jt, jsz, nch, cell):
                    ps = psA.tile([128, 512], F32, tag="psA", name="ps_o")
                    for ft in range(KT):
                        nc.tensor.matmul(
                            ps[:jsz, :384],
                            lhsT=attnT[:, ft, jt * 128:jt * 128 + jsz],
                            rhs=woT[:, ft, nch * 384:(nch + 1) * 384],
                            start=(ft == 0), stop=(ft == KT - 1))
                    dst = cell[0][:jsz, nch * 384:(nch + 1) * 384]
                    if seam:
                        nc.scalar.copy(dst, ps[:jsz, :384])
                        nc.gpsimd.tensor_tensor(
                            dst, dst,
                            bias_bc[:jsz, nch * 384:(nch + 1) * 384],
                            op=ALU.add)
                    else:
                        # PSUM->SBUF copy with the bias add fused in (DVE)
                        nc.vector.tensor_tensor(
                            dst, ps[:jsz, :384],
                            bias_bc[:jsz, nch * 384:(nch + 1) * 384],
                            op=ALU.add)

                def blob0(jt=jt, jsz=jsz, cell=cell):
                    cell.append(osp.tile([128, D], F32, tag="osb", name="osb"))
                    half(jt, jsz, 0, cell)

                def blob1(jt=jt, jsz=jsz, cell=cell):
                    half(jt, jsz, 1, cell)
                    # one fully-contiguous [jsz, 768] store per (item, j-tile)
                    nc.sync.dma_start(out_d[b, jt * 128:jt * 128 + jsz, :],
                                      cell[0][:jsz])
                blobs += [blob0, blob1]
            return blobs

        def transpose_attn(toks):
            """One xbar transpose pass: token-major attn [i, f] (two row
            tiles, the second padded to 80 rows) -> feature-major attn^T
            [f-part, kt, i] for the output projection. Columns 197..207
            receive garbage from the pad rows and are never read."""
            attnT = atp.tile([128, KT, N_PAD], BF16, tag="attnT",
                             name="attnT")
            # issued on the sync queue: a DMA issued from Act would hold
            # the Act SEQ during the HWDGE queue wait and delay the exps.
            # With the out-projection deferred a full cycle, the sync
            # queue's latency is well inside the transpose's slack.
            nc.sync.dma_start_transpose(attnT[:, :, 0:128], toks[0])
            nc.sync.dma_start_transpose(attnT[:, :, 128:N_PAD],
                                        toks[1][0:80])
            return attnT

        def emit_attn_weave(b, qk_t, fill_proj, fill_out, sprinkle):
            """Attention for item b, software-pipelined: filler blobs are
            popped between the S and PV matmuls of each head pair so the
            in-order PE stream never waits on the Act/DVE softmax chain.
            `sprinkle` holds staging jobs (DMA-side) to emit along the way."""

            def pop(dq1, dq2):
                if dq1:
                    dq1.popleft()()
                elif dq2:
                    dq2.popleft()()

            meng = nc.gpsimd if MASK_ON_POOL else nc.vector
            HP = H // 2
            itiles = [(0, 128), (1, N - 128)]
            pts, ots = {}, {}
            # TOKEN-MAJOR PV: out[i, d] = P^T.T @ V_aug puts query tokens on
            # the PSUM partitions, so the softmax row-sum (65th column) is a
            # PER-PARTITION scalar: normalize is a [jsz,1] reciprocal + a
            # per-partition tensor_scalar multiply — no partition broadcast,
            # no wide multiplies. The chain is stage-lagged (every engine
            # stream is in-order, so a sem-wait at the queue head stalls all
            # later ops): S/exp @ s, mask @ s (after a filler), PV @ s+1,
            # recip+mul @ s+1 (after a filler). attn lands token-major in
            # SBUF; one xbar transpose per item (next cycle) rebuilds the
            # feature-major attn^T that the output projection consumes.
            toks = [akp.tile([128, D], BF16, tag="tok", name="tok")
                    for _ in range(2)]
            for s in range(HP + 1):
                if s < HP:
                    # All four of the head pair's S^T matmuls (2 j-tiles x
                    # 2 heads) into one 4-bank tile, then ONE exp covering
                    # every slot (rows >= jsz of the j-tile-1 slots hold
                    # garbage no consumer reads).
                    st4 = psS.tile([128, 4, N], F32, tag="psS", name="st4",
                                   padded_shape=[128, 4, 512])
                    for jt, jsz in jtiles:
                        for hh in range(2):
                            pb = hh * 64
                            qa = qk_t[pb:pb + 64, s, :]
                            ka = qk_t[pb:pb + 64, 6 + s, :]
                            nc.tensor.matmul(
                                st4[:jsz, 2 * jt + hh],
                                lhsT=ka[:, jt * 128:jt * 128 + jsz],
                                rhs=qa, start=True, stop=True)
                    pt4 = ptp.tile([128, 4, N], BF16, tag="pt", name="pt4")
                    nc.scalar.activation(pt4, st4, AF.Exp)
                    pts[s] = pt4
                def pv_norm(hp, hh):
                    # PV + normalize for one head, both i-chunks, with the
                    # normalize (tiny DVE recip + per-partition multiply)
                    # emitted immediately so each 1-bank psO tile frees
                    # within half a stage (only 2 bufs exist).
                    h = 2 * hp + hh
                    for ic, icsz in itiles:
                        ot = psO.tile([128, 512], F32, tag="psO", name="ot")
                        for jt, jsz in jtiles:
                            nc.tensor.matmul(
                                ot[:icsz, :65],
                                lhsT=pts[hp][:jsz, 2 * jt + hh,
                                             ic * 128:ic * 128 + icsz],
                                rhs=v_sb[:jsz, b, jt, h * 65:h * 65 + 65],
                                start=(jt == 0), stop=(jt == 1))
                        rc = rcp.tile([128, 1], F32, tag="rc", name="rc")
                        nc.vector.reciprocal(rc[:icsz], ot[:icsz, 64:65])
                        nc.vector.tensor_scalar_mul(
                            toks[ic][:icsz, h * 64:h * 64 + 64],
                            ot[:icsz, 0:64], rc[:icsz])

                if 1 <= s:
                    pv_norm(s - 1, 0)
                pop(fill_proj, fill_out)
                if s < HP:
                    # ONE mask multiply over all 4 slots (layout matches)
                    meng.tensor_tensor(pts[s], pts[s], mask_sb, op=ALU.mult)
                if 1 <= s:
                    pv_norm(s - 1, 1)
                    pts.pop(s - 1, None)
                if s >= 2:
                    pop(fill_out, fill_proj)
                else:
                    pop(fill_proj, fill_out)
                pop(fill_proj, fill_out)
                if sprinkle:
                    sprinkle.popleft()()
            return toks

        def make_stage():
            """Allocate one repeat's staged tiles and build its 19 staging
            job emitters (not yet emitted)."""
            tiles = {
                "xT": xTp.tile([128, KT, M_pad], BF16, tag="xT", name="xT"),
                "wqkT": wqp.tile([128, KT, 2 * D], BF16, tag="wqkT",
                                 name="wqkT"),
                "wvT": wvp.tile([128, KT, D], BF16, tag="wvT", name="wvT"),
                "woT": wop.tile([128, KT, D], BF16, tag="woT", name="woT"),
            }
            jobs = {
                "x": pair_jobs(lambda r0, rows: x_flat[r0:r0 + rows, :],
                               M, lambda r0: tiles["xT"][:, :, r0:r0 + 128]),
                "wqk": pair_jobs(lambda r0, rows: wqkv_d[r0:r0 + rows, :],
                                 2 * D,
                                 lambda r0: tiles["wqkT"][:, :, r0:r0 + 128]),
                "wv": pair_jobs(
                    lambda r0, rows: wqkv_d[2 * D + r0:2 * D + r0 + rows, :],
                    D, lambda r0: tiles["wvT"][:, :, r0:r0 + 128]),
                "wout": pair_jobs(lambda r0, rows: wout_d[r0:r0 + rows, :],
                                  D, lambda r0: tiles["woT"][:, :, r0:r0 + 128]),
            }
            return tiles, jobs

        def emit_rep_body(tiles, own_jobs, next_jobs, carried=()):
            """One repeat's compute. own_jobs is set only for the FIRST
            repeat (cold staging interleaved with the prologue); later
            repeats find their tiles already staged, because each repeat
            sprinkles the NEXT repeat's staging jobs through its cycles
            (the sync SEQ is in-order, so only emission-time interleaving
            can overlap staging DMA with the previous repeat's compute)."""
            cur.clear()
            cur.update(tiles)
            qk_tiles = {}

            def qk_tile(b):
                if b not in qk_tiles:
                    qk_tiles[b] = qkp.tile([128, NT_QK, N], BF16, tag="qk",
                                           name="qk_t")
                return qk_tiles[b]

            qk0 = qkT_item_blobs(0, qk_tile(0))
            v0 = v_item_blobs(0)

            def run_job(j):
                j[0]()
                j[1]()

            if own_jobs is not None:
                # ---- cold prologue: stage and compute hand in hand ----
                xjobs, wqkjobs = own_jobs["x"], own_jobs["wqk"]
                wvjobs, woutjobs = own_jobs["wv"], own_jobs["wout"]
                run_job(wqkjobs[0])
                run_job(xjobs[0])
                run_job(xjobs[1])
                qk0[0]()
                qk0[1]()
                for j in range(1, 6):
                    run_job(wqkjobs[j])  # cols 256j..-: feature tiles 2j,2j+1
                    qk0[2 * j]()
                    qk0[2 * j + 1]()
                run_job(wvjobs[0])
                run_job(wvjobs[1])  # wv rows 0..511: nch0 ready
                v0[0]()
                v0[1]()
                run_job(wvjobs[2])  # nch1 ready
                run_job(xjobs[2])
                v0[2]()
                v0[3]()
                run_job(xjobs[3])
                own_left = [xjobs[4], *woutjobs, *xjobs[5:]]
            else:
                # steady prologue: interleave the previous repeat's carried
                # tail out-projections between this repeat's first blobs so
                # neither side drains the PE alone at the repeat seam.
                carry = deque(carried)
                for i, f in enumerate(qk0 + v0):
                    f()
                    if i % 2 == 1 and carry:
                        carry.popleft()()
                while carry:
                    carry.popleft()()
                own_left = []
            # sprinkle: leftover cold staging first, then the next repeat's
            # staging jobs. Loads run LAG slots ahead of their convert +
            # transpose phase so no engine queue ever parks on a DMA.
            nxt = []
            if next_jobs is not None:
                nxt = [*next_jobs["wqk"], *next_jobs["x"][:2],
                       *next_jobs["wv"], *next_jobs["x"][2:],
                       *next_jobs["wout"]]
            todo = own_left + nxt
            LAG = 2
            flat = []
            pending = deque()
            for ld, rs in todo:
                flat.append(ld)
                pending.append(rs)
                if len(pending) > LAG:
                    flat.append(pending.popleft())
            flat.extend(pending)
            sprinkle = deque(flat)
            # the out-projection of item b runs in cycle b+2: its attn^T
            # transpose is issued in cycle b+1 and so has a full cycle of
            # DMA-queue slack before the projection matmuls consume it.
            prev_toks = None
            attnTs = {}
            for b in range(bpc):
                fp = deque()
                if b + 1 < bpc:
                    fp.extend(qkT_item_blobs(b + 1, qk_tile(b + 1)))
                    fp.extend(v_item_blobs(b + 1))
                fo = deque()
                if b >= 1:
                    attnTs[b - 1] = transpose_attn(prev_toks)
                if b >= 2:
                    fo.extend(outproj_blobs(b - 2, attnTs.pop(b - 2),
                                            cur["woT"]))
                prev_toks = emit_attn_weave(b, qk_tile(b), fp, fo, sprinkle)
                for f in fp:
                    f()
                for f in fo:
                    f()
            attnTs[bpc - 1] = transpose_attn(prev_toks)
            tail = []
            for b in (bpc - 2, bpc - 1):
                tail.extend(outproj_blobs(b, attnTs.pop(b), cur["woT"],
                                          seam=True))
            return tail

        emit_preamble()
        tiles, jobs = make_stage()
        own = jobs
        tail = ()
        for _rep in range(repeat):
            if _rep + 1 < repeat:
                ntiles, njobs = make_stage()
            else:
                ntiles, njobs = None, None
            tail = emit_rep_body(tiles, own, njobs, carried=tail)
            tiles, own = ntiles, None
        for f in tail:
            f()

    nc.compile()
    return nc


_NC_CACHE = {}


def _get_nc(bpc=BPC, repeat=1):
    key = (bpc, repeat)
    if key not in _NC_CACHE:
        _NC_CACHE[key] = build_nc(bpc, repeat)
    return _NC_CACHE[key]


def kernel(x, W_qkv, scale, W_out, b_out, _trace=False):
    x = np.ascontiguousarray(np.asarray(x, dtype=np.float32))
    W_qkv = np.ascontiguousarray(np.asarray(W_qkv, dtype=np.float32))
    scale = np.ascontiguousarray(np.asarray(scale, dtype=np.float32))
    W_out = np.ascontiguousarray(np.asarray(W_out, dtype=np.float32))
    b_out = np.ascontiguousarray(np.asarray(b_out, dtype=np.float32))

    nc = _get_nc()
    in_maps = [{
        "x": x[c * BPC:(c + 1) * BPC],
        "w_qkv": W_qkv,
        "scale": scale,
        "w_out": W_out,
        "b_out": b_out,
    } for c in range(NCORES)]
    try:
        res = bass_utils.run_bass_kernel_spmd(
            nc, in_maps, core_ids=list(range(NCORES)), trace=_trace)
    except ModuleNotFoundError:
        # axon NTFF profiling hook unavailable in this container
        res = bass_utils.run_bass_kernel_spmd(
            nc, in_maps, core_ids=list(range(NCORES)), trace=False)
    out = np.concatenate([res.results[c]["out"] for c in range(NCORES)], axis=0)
    if _trace:
        return out, res
    return out
